# revision 6
# baseline (speedup 1.0000x reference)
"""Distributed Trainium2 attention kernel (8 NeuronCores).

Strategy: tensor-parallel over heads for QKV projection + attention
(4 query heads + their 1 shared KV head per core), then an AllToAll
switches to row-sharding so each core computes the output projection for
its 512 rows with the full wo. Host reassembles rows. All matmuls run in
bf16 with fp32 PSUM accumulation.

The PE sustains ~0.5 ns/moving-column (2.0 GHz effective under the
chip-wide power state) with LDWEIGHTS and semaphore updates fully hidden,
so the design minimizes *streamed columns* and keeps every other engine
off the PE's critical path:

- Phase B (QKV+RoPE): x tiles stationary, wqkv moving; all of x-group
  0 and 1 plus the weights are DMA'd up front (the JIT issue scheme
  starved the PE for ~30us at startup via engine-FIFO head-of-line
  blocking).
- Phase C (attention, [keys, q] layout): score tiles for consecutive
  key-block PAIRS are packed contiguously into one 2-bank PSUM tile so
  ONE exp instruction covers both (halves ScalarE's ~300ns/op fixed
  cost; ScalarE is the phase C bottleneck). The softmax denominator is
  DVE/GpSimd-accumulated from the exp'd tiles (whole chains alternate
  engines; the adds of one chain are serial anyway). The chain tail is
  one ones-stationary rowsum matmul ([1,512] = denominators), a [1,512]
  DVE reciprocal, and the ones^T (x) recip broadcast matmul -- no PE
  transposes and no ScalarE involvement.
- Phase D (output projection): two 4-bank PSUM sets; each cg's last
  head-group (the heads whose AllToAll lands last) is deferred until
  after the NEXT cg's first 24 head-tiles are emitted, so the PE always
  has ~25us of runway while the final AllToAll is in flight.

RoPE is applied in row-major layout via a host-side even/odd column
permutation of wq/wk (rotation becomes contiguous half-block arithmetic),
then q/k are transposed to [head_dim, rows] on the TensorEngine.
"""

import numpy as np
import ml_dtypes

import concourse.bass as bass
import concourse.mybir as mybir
import concourse.tile as tile
from concourse import bacc
from concourse import bass_utils

B, S, D = 2, 2048, 4096
H, HKV, HD = 32, 8, 128
HD2 = HD // 2
NC = 8
HL = H // NC            # 4 local q heads per core
BS = B * S              # 4096 global rows
R = BS // NC            # 512 output rows per core
NRB = BS // 128         # 32 row blocks
NDT = D // 128          # 32 contraction tiles
SCALE = 1.0 / float(np.sqrt(HD))
BF = mybir.dt.bfloat16
F32 = mybir.dt.float32

PROFILE = False         # set by test.py for neuron-profile capture
TMPDIR = None           # set by test.py to keep the trace dir


def _emit(nc, tc, io):
    xT, wqkvT, woT, ccR, ssR, trim, iden, out = io

    engs3 = (nc.sync, nc.scalar, nc.gpsimd)

    with (
        tc.tile_pool(name="cbuf", bufs=1) as cbuf,
        tc.tile_pool(name="qbuf", bufs=1) as qbuf,
        tc.tile_pool(name="kvbuf", bufs=1) as kvbuf,
        tc.tile_pool(name="dram", bufs=1, space="DRAM") as dram,
        tc.tile_pool(name="ps", bufs=1, space="PSUM") as ps,
    ):
        # ---- long-lived SBUF state ----
        q_sb = qbuf.tile([128, HL * BS], BF, tag="q")     # col = h*4096 + row
        kT_sb = kvbuf.tile([128, BS], BF, tag="k")        # col = row
        v_sb = kvbuf.tile([128, BS], BF, tag="v")         # col = rb*128 + hd

        trim_sb = cbuf.tile([128, 128], F32, tag="tm")
        iden_sb = cbuf.tile([128, 128], BF, tag="idn")
        onec_sb = cbuf.tile([128, 1], BF, tag="onec")
        oner_sb = cbuf.tile([1, 128], BF, tag="oner")

        # one AllToAll per local head (fired as soon as that head's chains
        # drain) so phase D's inputs arrive progressively
        a2a_in = [dram.tile([BS // 4, R], BF, name=f"a2a_in{h}") for h in range(4)]
        a2a_out = [dram.tile([BS // 4, R], BF, name=f"a2a_out{h}") for h in range(4)]

        # ================= phase B: QKV projection + RoPE =================
        with (
            tc.tile_pool(name="wbuf", bufs=1) as wbuf,
            tc.tile_pool(name="xs", bufs=1) as xs,
            tc.tile_pool(name="cs", bufs=6) as cs,
            tc.tile_pool(name="ts", bufs=8) as ts,
        ):
            # resident QKV weights: col = dt*768 + [0:512 q | 512:640 k | 640:768 v]
            w_sb = wbuf.tile([128, NDT * 768], BF, tag="w")
            # x tiles: one [128, 4*512] quad covers 4 d-slices x 512 rows
            xg = [[None] * (NDT // 4) for _ in range(8)]

            def issue_xg(g, dq):
                t = xs.tile([128, 2048], BF, tag="x", bufs=16, name=f"x{g}_{dq}")
                src_ap = xT[dq * 512:(dq + 1) * 512, g * 512:(g + 1) * 512] \
                    .rearrange("(b p) c -> p b c", p=128)
                dst_ap = t[:].rearrange("p (b c) -> p b c", b=4)
                engs3[(g * 8 + dq + 1) % 3].dma_start(dst_ap, src_ap)
                xg[g][dq] = t

            def issue_w(dt):
                engs3[dt % 3].dma_start(
                    w_sb[:, dt * 768: dt * 768 + 768],
                    wqkvT[dt * 128: (dt + 1) * 128, :],
                )

            nc.sync.dma_start(trim_sb[:], trim[:])
            nc.scalar.dma_start(iden_sb[:], iden[:])
            nc.vector.memset(onec_sb[:], 1.0)
            nc.vector.memset(oner_sb[:], 1.0)
            # Everything the first two row-groups need is issued up front
            # with no buffer-recycle waits at the head of any engine FIFO:
            # first 4 weight tiles, then x-group 0 interleaved with the
            # remaining weights, then x-group 1 (xs has exactly 16 slots).
            for dt in range(4):
                issue_w(dt)
            issue_xg(0, 0)
            nxt = 4
            for dq in range(1, 8):
                issue_xg(0, dq)
                for _ in range(3):
                    if nxt < NDT:
                        issue_w(nxt)
                        nxt += 1
            while nxt < NDT:
                issue_w(nxt)
                nxt += 1

            # rope tables: one [128, 1024] tile covers 4 row blocks
            csq = {}

            def issue_cs(q):
                cct = cs.tile([128, 1024], BF, tag="cc", bufs=3, name=f"cc{q}")
                engs3[q % 3].dma_start(cct[:], ccR[:, q * 1024: (q + 1) * 1024])
                sst = cs.tile([128, 1024], BF, tag="ss", bufs=3, name=f"ss{q}")
                engs3[(q + 1) % 3].dma_start(sst[:], ssR[:, q * 1024: (q + 1) * 1024])
                csq[q] = (cct, sst)

            issue_cs(0)
            for dq in range(8):
                issue_xg(1, dq)

            # rope tails are emitted one rb late, behind rb+1's matmuls
            def b_rope_tail_q(rb, ps_q):
                if rb % 4 == 0 and rb // 4 + 1 < 8:
                    issue_cs(rb // 4 + 1)
                cq, sq = csq[rb // 4]
                cct = cq[:, (rb % 4) * 256: (rb % 4 + 1) * 256]
                sst = sq[:, (rb % 4) * 256: (rb % 4 + 1) * 256]

                qe = ps_q[:].rearrange("p (h d) -> p h d", d=128)[:, :, 0:HD2]
                qo = ps_q[:].rearrange("p (h d) -> p h d", d=128)[:, :, HD2:HD]
                t1 = ts.tile([128, 256], BF, tag="t")
                t2 = ts.tile([128, 256], BF, tag="t")
                t3 = ts.tile([128, 256], BF, tag="t")
                t4 = ts.tile([128, 256], BF, tag="t")
                nc.vector.tensor_mul(t1[:], qe, cct)
                nc.vector.tensor_mul(t2[:], qo, sst)
                nc.vector.tensor_mul(t3[:], qe, sst)
                nc.vector.tensor_mul(t4[:], qo, cct)
                qrot = ts.tile([128, 512], BF, tag="qr")
                qre = qrot[:].rearrange("p (h d) -> p h d", d=128)[:, :, 0:HD2]
                qro = qrot[:].rearrange("p (h d) -> p h d", d=128)[:, :, HD2:HD]
                nc.vector.tensor_sub(qre, t1[:], t2[:])
                nc.vector.tensor_add(qro, t3[:], t4[:])
                return (qrot, cct, sst)

            def b_transpose_tail_q(rb, qrot):
                ps_tq = ps.tile([128, 512], BF, tag="aux", bufs=1, padded_shape=[128, 1024])
                for h in range(HL):
                    nc.tensor.transpose(
                        ps_tq[:, h * 128: (h + 1) * 128],
                        qrot[:, h * 128: (h + 1) * 128],
                        iden_sb[:],
                    )
                q_dst = (
                    q_sb[:]
                    .rearrange("p (h r) -> p h r", h=HL)
                    [:, :, rb * 128: (rb + 1) * 128]
                )
                nc.vector.tensor_copy(
                    q_dst, ps_tq[:].rearrange("p (h r) -> p h r", h=HL)
                )

            def b_rope_tail_kv(rb, ps_kv, cct, sst):
                ke = ps_kv[:, 0:HD2]
                ko = ps_kv[:, HD2:HD]
                u1 = ts.tile([128, 64], BF, tag="u")
                u2 = ts.tile([128, 64], BF, tag="u")
                u3 = ts.tile([128, 64], BF, tag="u")
                u4 = ts.tile([128, 64], BF, tag="u")
                nc.vector.tensor_mul(u1[:], ke, cct[:, 0:HD2])
                nc.vector.tensor_mul(u2[:], ko, sst[:, 0:HD2])
                nc.vector.tensor_mul(u3[:], ke, sst[:, 0:HD2])
                nc.vector.tensor_mul(u4[:], ko, cct[:, 0:HD2])

                krot = ts.tile([128, 128], BF, tag="kr")
                nc.vector.tensor_sub(krot[:, 0:HD2], u1[:], u2[:])
                nc.vector.tensor_add(krot[:, HD2:HD], u3[:], u4[:])

                # v: plain copy to row-major storage
                nc.scalar.activation(
                    v_sb[:, rb * 128: (rb + 1) * 128], ps_kv[:, 128:256],
                    mybir.ActivationFunctionType.Copy,
                )
                return (krot,)

            def b_transpose_tail_kv(rb, krot):
                ps_tk = ps.tile([128, 128], BF, tag="aux", bufs=1, padded_shape=[128, 1024])
                nc.tensor.transpose(ps_tk[:], krot[:], iden_sb[:])
                nc.vector.tensor_copy(kT_sb[:, rb * 128: (rb + 1) * 128], ps_tk[:])

            pending = None
            rot = None
            for rb in range(NRB):
                g, ri = rb // 4, rb % 4
                ps_q = ps.tile([128, 512], F32, tag="pa", bufs=3)
                ps_kv = ps.tile([128, 256], F32, tag="s2", bufs=2, padded_shape=[128, 1024])
                for dt in range(NDT):
                    xt = xg[g][dt // 4][:, (dt % 4) * 512 + ri * 128:
                                        (dt % 4) * 512 + (ri + 1) * 128]
                    st, sp = dt == 0, dt == NDT - 1
                    nc.tensor.matmul(
                        ps_q[:], xt, w_sb[:, dt * 768: dt * 768 + 512],
                        start=st, stop=sp,
                    )
                    nc.tensor.matmul(
                        ps_kv[:], xt, w_sb[:, dt * 768 + 512: dt * 768 + 768],
                        start=st, stop=sp,
                    )
                    # prefetch next row-group's x quads, spread over this group
                    # (groups 0 and 1 were fully issued up front)
                    if ri == 2 and g >= 1 and g + 1 < 8 and dt % 4 == 1:
                        issue_xg(g + 1, dt // 4)
                    if dt == 2 and pending is not None:
                        pq = b_rope_tail_q(pending[0], pending[1])
                        pkv = b_rope_tail_kv(pending[0], pending[2], pq[1], pq[2])
                        rot = (pending[0], pq[0]) + pkv
                        pending = None
                    if dt == 12 and rot is not None:
                        b_transpose_tail_q(rot[0], rot[1])
                        b_transpose_tail_kv(rot[0], rot[2])
                        rot = None
                pending = (rb, ps_q, ps_kv)
            pq = b_rope_tail_q(pending[0], pending[1])
            pkv = b_rope_tail_kv(pending[0], pending[2], pq[1], pq[2])
            b_transpose_tail_q(pending[0], pq[0])
            b_transpose_tail_kv(pending[0], pkv[0])

        # ============ phase C: causal attention (flipped PV) ============
        with (
            tc.tile_pool(name="es", bufs=6) as es,
            tc.tile_pool(name="rns", bufs=4) as rns,
            tc.tile_pool(name="abuf", bufs=1) as abuf,
            tc.tile_pool(name="ws", bufs=1) as ws,
            tc.tile_pool(name="osp", bufs=4) as osp,
        ):
            at_sb = abuf.tile([128, 32 * 512], BF, tag="at")  # col = ht*512+row
            # head-major so the earliest AllToAlls feed phase D's first
            # accumulation steps
            ht_order = [4 * i + l for l in range(4) for i in range(8)]
            wt0 = {}  # prefetched wo tiles for cg 0

            if True:

                def head_done(h):
                    """Fire head h's AllToAll + phase-D prefetches."""
                    nc.gpsimd.collective_compute(
                        "AllToAll",
                        mybir.AluOpType.bypass,
                        replica_groups=[list(range(NC))],
                        ins=[a2a_in[h].opt()],
                        outs=[a2a_out[h].opt()],
                    )
                    dst_ap = at_sb[:].rearrange(
                        "p (i c) -> p i c", c=512
                    )[:, h::4, :]
                    src_ap = a2a_out[h][:].rearrange("(i p) c -> p i c", p=128)
                    nc.sync.dma_start(dst_ap, src_ap)
                    if h == 0:
                        for k in range(0, 32, 4):
                            wt = ws.tile([128, 2048], BF, tag="wo", bufs=16,
                                         name=f"wt0_{k}")
                            i0, lv = k % 8, k // 8
                            src_ap = woT[:].rearrange(
                                "(a l p) c -> p a l c", p=128, l=4
                            )[:, i0: i0 + 4, lv, 0:512]
                            nc.sync.dma_start(
                                wt[:].rearrange("p (b c) -> p b c", b=4), src_ap
                            )
                            for n, ht in enumerate(ht_order[k: k + 4]):
                                wt0[ht] = wt[:, n * 512: (n + 1) * 512]

                def attn_chain(b, h, ci):
                    # pair-packed j pipeline: two key blocks' score tiles land
                    # contiguously in one 2-bank PSUM tile -> one exp, then PV
                    # (v-stationary, [hd, q] PSUM accumulation) with the
                    # softmax denominator accumulated from the exp'd tiles
                    # (vector/gpsimd alternating by chain).
                    qbase = h * BS + b * S
                    jmax = 4 * ci + 3
                    acc_eng = nc.vector if (2 * b + h) % 2 == 0 else nc.gpsimd
                    ps_attn = ps.tile([128, 512], F32, tag="pa", bufs=3,
                                       name=f"pa{b}{h}{ci}")
                    acc = rns.tile([128, 512], BF, tag="acc", bufs=3,
                                   name=f"acc{b}{h}{ci}")

                    def pv_and_acc(p, et2, specs):
                        for (j, off, w, qo) in specs:
                            kcol = (b * 16 + j) * 128
                            nc.tensor.matmul(
                                ps_attn[:, qo: qo + w],
                                v_sb[:, kcol: kcol + 128],
                                et2[:, off: off + w],
                                start=(j == 0), stop=(j == jmax),
                                skip_group_check=True,
                            )
                        (ja, offa, wa, qoa), (jb, offb, wb, qob) = specs
                        if p == 0:
                            nc.vector.tensor_copy(
                                acc[:, qoa: qoa + wa], et2[:, offa: offa + wa]
                            )
                            nc.vector.tensor_add(
                                acc[:, qob: qob + wb], acc[:, qob: qob + wb],
                                et2[:, offb: offb + wb],
                            )
                        else:
                            acc_eng.tensor_add(
                                acc[:, qoa: qoa + wa], acc[:, qoa: qoa + wa],
                                et2[:, offa: offa + wa],
                            )
                            acc_eng.tensor_add(
                                acc[:, qob: qob + wb], acc[:, qob: qob + wb],
                                et2[:, offb: offb + wb],
                            )

                    prev = None
                    for p in range(2 * ci + 2):
                        ja, jb = 2 * p, 2 * p + 1
                        q0a = max(ja * 128, 512 * ci)
                        wa = 512 * ci + 512 - q0a
                        q0b = max(jb * 128, 512 * ci)
                        wb = 512 * ci + 512 - q0b
                        qoa, qob = q0a - 512 * ci, q0b - 512 * ci
                        pp = ps.tile([128, 1024], F32, tag="s2", bufs=2,
                                      name=f"s{b}{h}{ci}_{p}")
                        nc.tensor.matmul(
                            pp[:, 0:wa],
                            kT_sb[:, (b * 16 + ja) * 128: (b * 16 + ja) * 128 + 128],
                            q_sb[:, qbase + q0a: qbase + q0a + wa],
                            start=True, stop=True, skip_group_check=True,
                        )
                        # region b needs its own has_written clear only when
                        # it lands in bank 1; when it shares bank 0 with
                        # region a (wa < 512), region a's start already
                        # cleared its bits and a second clear is not needed.
                        nc.tensor.matmul(
                            pp[:, wa: wa + wb],
                            kT_sb[:, (b * 16 + jb) * 128: (b * 16 + jb) * 128 + 128],
                            q_sb[:, qbase + q0b: qbase + q0b + wb],
                            start=(wa == 512), stop=True, skip_group_check=True,
                        )
                        if ja >= 4 * ci:
                            nc.vector.tensor_add(
                                pp[:, 0:128], pp[:, 0:128], trim_sb[:]
                            )
                        if jb >= 4 * ci:
                            nc.vector.tensor_add(
                                pp[:, wa: wa + 128], pp[:, wa: wa + 128],
                                trim_sb[:],
                            )
                        et2 = es.tile([128, 1024], BF, tag="e", bufs=6,
                                      name=f"e{b}{h}{ci}_{p}")
                        nc.scalar.activation(
                            et2[:, 0: wa + wb], pp[:, 0: wa + wb],
                            mybir.ActivationFunctionType.Exp, scale=SCALE,
                        )
                        if prev is not None:
                            pv_and_acc(*prev)
                        prev = (p, et2,
                                ((ja, 0, wa, qoa), (jb, wa, wb, qob)))
                        yield
                    pv_and_acc(*prev)

                    # tail: ones-stationary rowsum matmul -> [1,512]
                    # denominators in PSUM, [1,512] DVE reciprocal, then the
                    # ones^T (x) recip broadcast matmul and one normalizing
                    # multiply.
                    psr = ps.tile([1, 512], F32, tag="aux", bufs=1,
                                   padded_shape=[128, 512],
                                   name=f"rs{b}{h}{ci}")
                    nc.tensor.matmul(
                        psr[0:1, :], onec_sb[:], acc[:],
                        start=True, stop=True, skip_group_check=True,
                    )
                    rc_row = rns.tile([1, 512], BF, tag="rcrow", bufs=2)
                    with nc.allow_low_precision(
                        reason="softmax reciprocal consumed as bf16 matmul "
                               "operand; matches baseline precision"
                    ):
                        nc.vector.reciprocal(rc_row[:], psr[:])
                    bc_ps = ps.tile([128, 512], F32, tag="aux", bufs=1,
                                     name=f"bc{b}{h}{ci}")
                    nc.tensor.matmul(
                        bc_ps[:], oner_sb[:], rc_row[:],
                        start=True, stop=True, skip_group_check=True,
                    )
                    bc = rns.tile([128, 512], F32, tag="bc", bufs=2)
                    nc.vector.tensor_copy(bc[:], bc_ps[:])
                    an = rns.tile([128, 512], BF, tag="an")
                    nc.vector.tensor_mul(an[:], ps_attn[:], bc[:])
                    nc.sync.dma_start(
                        a2a_in[h][128 * (b * 4 + ci): 128 * (b * 4 + ci) + 128, :],
                        an[:],
                    )
                    yield

                # continuous 2-in-flight worklist; fire each head's AllToAll
                # the moment its last chain drains
                todo = [(b, h, ci)
                        for h in range(4) for b in range(B)
                        for ci in (0, 3, 1, 2)]
                todo.reverse()
                left = {h: 2 * 4 for h in range(4)}
                active = [[todo[-1][1], attn_chain(*todo.pop())],
                          [todo[-1][1], attn_chain(*todo.pop())],
                          [todo[-1][1], attn_chain(*todo.pop())]]
                while active:
                    for ent in list(active):
                        if next(ent[1], StopIteration) is StopIteration:
                            active.remove(ent)
                            left[ent[0]] -= 1
                            if left[ent[0]] == 0:
                                head_done(ent[0])
                            if todo:
                                active.append(
                                    [todo[-1][1], attn_chain(*todo.pop())]
                                )

            # ======== phase D: output projection for this core's rows ========
            # Two 4-bank PSUM sets; each cg's k=6,7 head-groups (the heads
            # whose AllToAll lands last) are deferred until after the next
            # cg's k=0..5, so the final AllToAll is fully hidden.
            if True:
                wtq = [None] * 8  # per-cg {ht: wt slice}

                def d_load_wt(cg, k):
                    wq4 = ws.tile([128, 2048], BF, tag="wo",
                                  bufs=16, name=f"wt{cg}_{k}")
                    i0, lv = 4 * (k % 2), k // 2
                    src_ap = woT[:].rearrange(
                        "(a l p) c -> p a l c", p=128, l=4
                    )[:, i0: i0 + 4, lv,
                      cg * 512: (cg + 1) * 512]
                    engs3[k % 3].dma_start(
                        wq4[:].rearrange("p (b c) -> p b c", b=4),
                        src_ap,
                    )
                    for n, ht2 in enumerate(ht_order[4 * k: 4 * k + 4]):
                        wtq[cg][ht2] = wq4[:, n * 512: (n + 1) * 512]

                po_sets = [None] * 8

                def d_alloc(cg):
                    if cg % 2 == 0:
                        t = [ps.tile([128, 512], F32, tag="pa", bufs=3,
                                     name=f"po{cg}_{i}")[:] for i in range(3)]
                        t.append(ps.tile([128, 512], F32, tag="aux", bufs=1,
                                         name=f"po{cg}_3")[:])
                        return t
                    s0 = ps.tile([128, 1024], F32, tag="s2", bufs=2,
                                 name=f"po{cg}_01")
                    s1 = ps.tile([128, 1024], F32, tag="s2", bufs=2,
                                 name=f"po{cg}_23")
                    return [s0[:, 0:512], s0[:, 512:1024],
                            s1[:, 0:512], s1[:, 512:1024]]

                def d_emit(cg, ks):
                    po = po_sets[cg]
                    for k in ks:
                        if cg == 0:
                            wt4 = [wt0[ht] for ht in ht_order[4 * k: 4 * k + 4]]
                        else:
                            d_load_wt(cg, k)
                            wt4 = [wtq[cg][ht] for ht in ht_order[4 * k: 4 * k + 4]]
                        for n, ht in enumerate(ht_order[4 * k: 4 * k + 4]):
                            n_ht = 4 * k + n
                            for rt in range(4):
                                nc.tensor.matmul(
                                    po[rt],
                                    at_sb[:, ht * 512 + rt * 128:
                                          ht * 512 + (rt + 1) * 128],
                                    wt4[n],
                                    start=(n_ht == 0), stop=(n_ht == 31),
                                    skip_group_check=True,
                                )

                def d_copyout(cg):
                    for rt in range(4):
                        ot = osp.tile([128, 512], F32, tag="o")
                        nc.vector.tensor_copy(ot[:], po_sets[cg][rt])
                        engs3[rt % 2].dma_start(
                            out[rt * 128: (rt + 1) * 128,
                                cg * 512: (cg + 1) * 512],
                            ot[:],
                        )
                    po_sets[cg] = None

                for cg in range(9):
                    if cg < 8:
                        wtq[cg] = {}
                        po_sets[cg] = d_alloc(cg)
                        d_emit(cg, range(6))
                    if cg >= 1:
                        d_emit(cg - 1, range(6, 8))
                        d_copyout(cg - 1)


def _build():
    nc = bacc.Bacc("TRN2", target_bir_lowering=False, debug=False, num_devices=NC)
    xT = nc.dram_tensor("xT", [D, BS], BF, kind="ExternalInput")
    wqkvT = nc.dram_tensor("wqkvT", [D, 768], BF, kind="ExternalInput")
    woT = nc.dram_tensor("woT", [D, D], BF, kind="ExternalInput")
    ccR = nc.dram_tensor("ccR", [128, NRB * 256], BF, kind="ExternalInput")
    ssR = nc.dram_tensor("ssR", [128, NRB * 256], BF, kind="ExternalInput")
    trim = nc.dram_tensor("trim", [128, 128], F32, kind="ExternalInput")
    iden = nc.dram_tensor("iden", [128, 128], BF, kind="ExternalInput")
    out = nc.dram_tensor("out", [R, D], F32, kind="ExternalOutput")
    with tile.TileContext(nc) as tc:
        _emit(nc, tc, (xT, wqkvT, woT, ccR, ssR, trim, iden, out))
    nc.compile()
    return nc


_NC = None


def kernel(x, wq, wk, wv, wo, freqs_cos, freqs_sin, mask, start_pos):
    global _NC
    if _NC is None:
        _NC = _build()
    nc = _NC
    bf = ml_dtypes.bfloat16

    x = np.asarray(x, dtype=np.float32)
    xT = np.ascontiguousarray(x.reshape(BS, D).T).astype(bf)

    perm = np.concatenate([np.arange(0, HD, 2), np.arange(1, HD, 2)])
    wqTp = np.asarray(wq, np.float32).T.reshape(D, H, HD)[:, :, perm]
    wkTp = np.asarray(wk, np.float32).T.reshape(D, HKV, HD)[:, :, perm]
    wvT = np.asarray(wv, np.float32).T.reshape(D, HKV, HD)
    woT = np.ascontiguousarray(np.asarray(wo, np.float32).T).astype(bf)

    fc = np.asarray(freqs_cos, np.float32)
    fs = np.asarray(freqs_sin, np.float32)
    # row-major RoPE tables per row block, replicated x4 along free axis
    pos = (np.arange(BS) % S).reshape(NRB, 128)
    ccR = np.tile(fc[pos], (1, 1, 4)).transpose(1, 0, 2).reshape(128, NRB * 256)
    ssR = np.tile(fs[pos], (1, 1, 4)).transpose(1, 0, 2).reshape(128, NRB * 256)
    ccR = np.ascontiguousarray(ccR).astype(bf)
    ssR = np.ascontiguousarray(ssR).astype(bf)

    trim = np.where(
        np.arange(128)[:, None] > np.arange(128)[None, :], -1e30, 0.0
    ).astype(np.float32)
    iden = np.eye(128, dtype=bf)

    in_maps = []
    for c in range(NC):
        wqkv = np.concatenate(
            [
                wqTp[:, 4 * c: 4 * c + 4].reshape(D, 512),
                wkTp[:, c],
                wvT[:, c],
            ],
            axis=1,
        ).astype(bf)
        in_maps.append(
            {
                "xT": xT,
                "wqkvT": np.ascontiguousarray(wqkv),
                "woT": woT,
                "ccR": ccR,
                "ssR": ssR,
                "trim": trim,
                "iden": iden,
            }
        )

    res = bass_utils.run_bass_kernel_spmd(
        nc, in_maps, core_ids=list(range(NC)), trace=PROFILE, tmpdir=TMPDIR
    )
    if PROFILE:
        print(f"HW exec time: {res.exec_time_ns} ns")
        if res.instructions_and_trace is not None:
            print(f"trace: {res.instructions_and_trace[1]}")

    out_full = np.empty((BS, D), dtype=np.float32)
    for c in range(NC):
        out_full[R * c: R * (c + 1)] = res.results[c]["out"]
    return out_full.reshape(B, S, D)


# revision 9
# speedup vs baseline: 1.0927x; 1.0927x over previous
"""Distributed Trainium2 attention kernel (8 NeuronCores).

Strategy: tensor-parallel over heads for QKV projection + attention
(4 query heads + their 1 shared KV head per core), then an AllToAll
switches to row-sharding so each core computes the output projection for
its 512 rows with the full wo. Host reassembles rows. All matmuls run in
bf16 with fp32 PSUM accumulation.

The PE sustains ~0.5 ns/moving-column (2.0 GHz effective under the
chip-wide power state) with LDWEIGHTS and semaphore updates fully hidden,
so the design minimizes *streamed columns* and keeps every other engine
off the PE's critical path:

- Phase B (QKV+RoPE): x tiles stationary, wqkv moving; all of x-group
  0 and 1 plus the weights are DMA'd up front (the JIT issue scheme
  starved the PE for ~30us at startup via engine-FIFO head-of-line
  blocking).
- Phase C (attention, [keys, q] layout): score tiles for consecutive
  key-block PAIRS are packed contiguously into one 2-bank PSUM tile so
  ONE exp instruction covers both (halves ScalarE's ~300ns/op fixed
  cost; ScalarE is the phase C bottleneck). The softmax denominator is
  DVE/GpSimd-accumulated from the exp'd tiles (whole chains alternate
  engines; the adds of one chain are serial anyway). The chain tail is
  one ones-stationary rowsum matmul ([1,512] = denominators), a [1,512]
  DVE reciprocal, and the ones^T (x) recip broadcast matmul -- no PE
  transposes and no ScalarE involvement.
- Phase D (output projection): two 4-bank PSUM sets; each cg's last
  head-group (the heads whose AllToAll lands last) is deferred until
  after the NEXT cg's first 24 head-tiles are emitted, so the PE always
  has ~25us of runway while the final AllToAll is in flight.

RoPE is applied in row-major layout via a host-side even/odd column
permutation of wq/wk (rotation becomes contiguous half-block arithmetic),
then q/k are transposed to [head_dim, rows] on the TensorEngine.
"""

import numpy as np
import ml_dtypes

import concourse.bass as bass
import concourse.mybir as mybir
import concourse.tile as tile
from concourse import bacc
from concourse import bass_utils

B, S, D = 2, 2048, 4096
H, HKV, HD = 32, 8, 128
HD2 = HD // 2
NC = 8
HL = H // NC            # 4 local q heads per core
BS = B * S              # 4096 global rows
R = BS // NC            # 512 output rows per core
NRB = BS // 128         # 32 row blocks
NDT = D // 128          # 32 contraction tiles
SCALE = 1.0 / float(np.sqrt(HD))
BF = mybir.dt.bfloat16
F32 = mybir.dt.float32

PROFILE = False         # set by test.py for neuron-profile capture
TMPDIR = None           # set by test.py to keep the trace dir


def _emit(nc, tc, io):
    xT, wqkvT, woT, ccR, ssR, trim, iden, out = io

    engs3 = (nc.sync, nc.scalar, nc.gpsimd)

    with (
        tc.tile_pool(name="cbuf", bufs=1) as cbuf,
        tc.tile_pool(name="qbuf", bufs=1) as qbuf,
        tc.tile_pool(name="kvbuf", bufs=1) as kvbuf,
        tc.tile_pool(name="dram", bufs=1, space="DRAM") as dram,
        tc.tile_pool(name="ps", bufs=1, space="PSUM") as ps,
    ):
        # ---- long-lived SBUF state ----
        q_sb = qbuf.tile([128, HL * BS], BF, tag="q")     # col = h*4096 + row
        kT_sb = kvbuf.tile([128, BS], BF, tag="k")        # col = row
        v_sb = kvbuf.tile([128, BS], BF, tag="v")         # col = rb*128 + hd

        trim_sb = cbuf.tile([128, 128], F32, tag="tm")
        iden_sb = cbuf.tile([128, 128], BF, tag="idn")
        onec_sb = cbuf.tile([128, 1], BF, tag="onec")
        oner_sb = cbuf.tile([1, 128], BF, tag="oner")

        # one AllToAll per local head (fired as soon as that head's chains
        # drain) so phase D's inputs arrive progressively
        a2a_in = [dram.tile([BS // 4, R], BF, name=f"a2a_in{h}") for h in range(4)]
        a2a_out = [dram.tile([BS // 4, R], BF, name=f"a2a_out{h}") for h in range(4)]

        # ================= phase B: QKV projection + RoPE =================
        with (
            tc.tile_pool(name="wbuf", bufs=1) as wbuf,
            tc.tile_pool(name="xs", bufs=1) as xs,
            tc.tile_pool(name="cs", bufs=6) as cs,
            tc.tile_pool(name="ts", bufs=8) as ts,
        ):
            # resident QKV weights: col = dt*768 + [0:512 q | 512:640 k | 640:768 v]
            w_sb = wbuf.tile([128, NDT * 768], BF, tag="w")
            # x tiles: one [128, 4*512] quad covers 4 d-slices x 512 rows
            xg = [[None] * (NDT // 4) for _ in range(8)]

            def issue_xg(g, dq):
                t = xs.tile([128, 2048], BF, tag="x", bufs=16, name=f"x{g}_{dq}")
                src_ap = xT[dq * 512:(dq + 1) * 512, g * 512:(g + 1) * 512] \
                    .rearrange("(b p) c -> p b c", p=128)
                dst_ap = t[:].rearrange("p (b c) -> p b c", b=4)
                engs3[(g * 8 + dq + 1) % 3].dma_start(dst_ap, src_ap)
                xg[g][dq] = t

            def issue_w(dt):
                engs3[dt % 3].dma_start(
                    w_sb[:, dt * 768: dt * 768 + 768],
                    wqkvT[dt * 128: (dt + 1) * 128, :],
                )

            nc.sync.dma_start(trim_sb[:], trim[:])
            nc.scalar.dma_start(iden_sb[:], iden[:])
            nc.vector.memset(onec_sb[:], 1.0)
            nc.vector.memset(oner_sb[:], 1.0)
            # Everything the first two row-groups need is issued up front
            # with no buffer-recycle waits at the head of any engine FIFO:
            # first 4 weight tiles, then x-group 0 interleaved with the
            # remaining weights, then x-group 1 (xs has exactly 16 slots).
            for dt in range(4):
                issue_w(dt)
            issue_xg(0, 0)
            nxt = 4
            for dq in range(1, 8):
                issue_xg(0, dq)
                for _ in range(3):
                    if nxt < NDT:
                        issue_w(nxt)
                        nxt += 1
            while nxt < NDT:
                issue_w(nxt)
                nxt += 1

            # rope tables: one [128, 1024] tile covers 4 row blocks
            csq = {}

            def issue_cs(q):
                cct = cs.tile([128, 1024], BF, tag="cc", bufs=3, name=f"cc{q}")
                engs3[q % 3].dma_start(cct[:], ccR[:, q * 1024: (q + 1) * 1024])
                sst = cs.tile([128, 1024], BF, tag="ss", bufs=3, name=f"ss{q}")
                engs3[(q + 1) % 3].dma_start(sst[:], ssR[:, q * 1024: (q + 1) * 1024])
                csq[q] = (cct, sst)

            issue_cs(0)
            for dq in range(8):
                issue_xg(1, dq)

            # rope tails are emitted one rb late, behind rb+1's matmuls
            def b_rope_tail_q(rb, ps_q):
                if rb % 4 == 0 and rb // 4 + 1 < 8:
                    issue_cs(rb // 4 + 1)
                cq, sq = csq[rb // 4]
                cct = cq[:, (rb % 4) * 256: (rb % 4 + 1) * 256]
                sst = sq[:, (rb % 4) * 256: (rb % 4 + 1) * 256]

                qe = ps_q[:].rearrange("p (h d) -> p h d", d=128)[:, :, 0:HD2]
                qo = ps_q[:].rearrange("p (h d) -> p h d", d=128)[:, :, HD2:HD]
                t1 = ts.tile([128, 256], BF, tag="t")
                t2 = ts.tile([128, 256], BF, tag="t")
                t3 = ts.tile([128, 256], BF, tag="t")
                t4 = ts.tile([128, 256], BF, tag="t")
                nc.vector.tensor_mul(t1[:], qe, cct)
                nc.vector.tensor_mul(t2[:], qo, sst)
                nc.vector.tensor_mul(t3[:], qe, sst)
                nc.vector.tensor_mul(t4[:], qo, cct)
                qrot = ts.tile([128, 512], BF, tag="qr")
                qre = qrot[:].rearrange("p (h d) -> p h d", d=128)[:, :, 0:HD2]
                qro = qrot[:].rearrange("p (h d) -> p h d", d=128)[:, :, HD2:HD]
                nc.vector.tensor_sub(qre, t1[:], t2[:])
                nc.vector.tensor_add(qro, t3[:], t4[:])
                return (qrot, cct, sst)

            def b_transpose_tail_q(rb, qrot):
                ps_tq = ps.tile([128, 512], BF, tag="aux", bufs=1, padded_shape=[128, 1024])
                for h in range(HL):
                    nc.tensor.transpose(
                        ps_tq[:, h * 128: (h + 1) * 128],
                        qrot[:, h * 128: (h + 1) * 128],
                        iden_sb[:],
                    )
                q_dst = (
                    q_sb[:]
                    .rearrange("p (h r) -> p h r", h=HL)
                    [:, :, rb * 128: (rb + 1) * 128]
                )
                nc.vector.tensor_copy(
                    q_dst, ps_tq[:].rearrange("p (h r) -> p h r", h=HL)
                )

            def b_rope_tail_kv(rb, ps_kv, cct, sst):
                ke = ps_kv[:, 0:HD2]
                ko = ps_kv[:, HD2:HD]
                u1 = ts.tile([128, 64], BF, tag="u")
                u2 = ts.tile([128, 64], BF, tag="u")
                u3 = ts.tile([128, 64], BF, tag="u")
                u4 = ts.tile([128, 64], BF, tag="u")
                nc.vector.tensor_mul(u1[:], ke, cct[:, 0:HD2])
                nc.vector.tensor_mul(u2[:], ko, sst[:, 0:HD2])
                nc.vector.tensor_mul(u3[:], ke, sst[:, 0:HD2])
                nc.vector.tensor_mul(u4[:], ko, cct[:, 0:HD2])

                krot = ts.tile([128, 128], BF, tag="kr")
                nc.vector.tensor_sub(krot[:, 0:HD2], u1[:], u2[:])
                nc.vector.tensor_add(krot[:, HD2:HD], u3[:], u4[:])

                # v: plain copy to row-major storage
                nc.scalar.activation(
                    v_sb[:, rb * 128: (rb + 1) * 128], ps_kv[:, 128:256],
                    mybir.ActivationFunctionType.Copy,
                )
                return (krot,)

            def b_transpose_tail_kv(rb, krot):
                ps_tk = ps.tile([128, 128], BF, tag="aux", bufs=1, padded_shape=[128, 1024])
                nc.tensor.transpose(ps_tk[:], krot[:], iden_sb[:])
                nc.vector.tensor_copy(kT_sb[:, rb * 128: (rb + 1) * 128], ps_tk[:])

            pending = None
            rot = None
            for rb in range(NRB):
                g, ri = rb // 4, rb % 4
                ps_q = ps.tile([128, 512], F32, tag="pa", bufs=3)
                ps_kv = ps.tile([128, 256], F32, tag="s2", bufs=2, padded_shape=[128, 1024])
                for dt in range(NDT):
                    xt = xg[g][dt // 4][:, (dt % 4) * 512 + ri * 128:
                                        (dt % 4) * 512 + (ri + 1) * 128]
                    st, sp = dt == 0, dt == NDT - 1
                    nc.tensor.matmul(
                        ps_q[:], xt, w_sb[:, dt * 768: dt * 768 + 512],
                        start=st, stop=sp,
                    )
                    nc.tensor.matmul(
                        ps_kv[:], xt, w_sb[:, dt * 768 + 512: dt * 768 + 768],
                        start=st, stop=sp,
                    )
                    # prefetch next row-group's x quads, spread over this group
                    # (groups 0 and 1 were fully issued up front)
                    if ri == 2 and g >= 1 and g + 1 < 8 and dt % 4 == 1:
                        issue_xg(g + 1, dt // 4)
                    if dt == 2 and pending is not None:
                        pq = b_rope_tail_q(pending[0], pending[1])
                        pkv = b_rope_tail_kv(pending[0], pending[2], pq[1], pq[2])
                        rot = (pending[0], pq[0]) + pkv
                        pending = None
                    if dt == 12 and rot is not None:
                        b_transpose_tail_q(rot[0], rot[1])
                        b_transpose_tail_kv(rot[0], rot[2])
                        rot = None
                pending = (rb, ps_q, ps_kv)
            pq = b_rope_tail_q(pending[0], pending[1])
            pkv = b_rope_tail_kv(pending[0], pending[2], pq[1], pq[2])
            b_transpose_tail_q(pending[0], pq[0])
            b_transpose_tail_kv(pending[0], pkv[0])

        # ============ phase C: causal attention (flipped PV) ============
        with (
            tc.tile_pool(name="es", bufs=6) as es,
            tc.tile_pool(name="rns", bufs=4) as rns,
            tc.tile_pool(name="abuf", bufs=1) as abuf,
            tc.tile_pool(name="ws", bufs=1) as ws,
            tc.tile_pool(name="osp", bufs=4) as osp,
        ):
            at_sb = abuf.tile([128, 32 * 512], BF, tag="at")  # col = ht*512+row
            # head-major so the earliest AllToAlls feed phase D's first
            # accumulation steps
            ht_order = [4 * i + l for l in range(4) for i in range(8)]
            wt0 = {}  # prefetched wo tiles for cg 0

            if True:

                def head_done(h):
                    """Fire head h's AllToAll + phase-D prefetches."""
                    nc.gpsimd.collective_compute(
                        "AllToAll",
                        mybir.AluOpType.bypass,
                        replica_groups=[list(range(NC))],
                        ins=[a2a_in[h].opt()],
                        outs=[a2a_out[h].opt()],
                    )
                    dst_ap = at_sb[:].rearrange(
                        "p (i c) -> p i c", c=512
                    )[:, h::4, :]
                    src_ap = a2a_out[h][:].rearrange("(i p) c -> p i c", p=128)
                    nc.gpsimd.dma_start(dst_ap, src_ap)
                    if h == 0:
                        for k in range(0, 32, 4):
                            wt = ws.tile([128, 2048], BF, tag="wo", bufs=16,
                                         name=f"wt0_{k}")
                            i0, lv = k % 8, k // 8
                            src_ap = woT[:].rearrange(
                                "(a l p) c -> p a l c", p=128, l=4
                            )[:, i0: i0 + 4, lv, 0:512]
                            nc.gpsimd.dma_start(
                                wt[:].rearrange("p (b c) -> p b c", b=4), src_ap
                            )
                            for n, ht in enumerate(ht_order[k: k + 4]):
                                wt0[ht] = wt[:, n * 512: (n + 1) * 512]

                def attn_chain(b, h, ci):
                    # pair-packed j pipeline: two key blocks' score tiles land
                    # contiguously in one 2-bank PSUM tile -> one exp, then PV
                    # (v-stationary, [hd, q] PSUM accumulation) with the
                    # softmax denominator accumulated from the exp'd tiles
                    # (vector/gpsimd alternating by chain).
                    qbase = h * BS + b * S
                    jmax = 4 * ci + 3
                    # the gpsimd queue blocks ~13us at each head boundary
                    # (collective + at-copy head the FIFO), so the chains
                    # that are active right then -- the first few of each
                    # head -- accumulate on the DVE instead
                    acc_eng = nc.gpsimd if (b == 1 or ci == 2) else nc.vector
                    ps_attn = ps.tile([128, 512], F32, tag="pa", bufs=3,
                                       name=f"pa{b}{h}{ci}")
                    acc = rns.tile([128, 512], BF, tag="acc", bufs=3,
                                   name=f"acc{b}{h}{ci}")

                    def pv_and_acc(p, et2, specs):
                        for (j, off, w, qo) in specs:
                            kcol = (b * 16 + j) * 128
                            nc.tensor.matmul(
                                ps_attn[:, qo: qo + w],
                                v_sb[:, kcol: kcol + 128],
                                et2[:, off: off + w],
                                start=(j == 0), stop=(j == jmax),
                                skip_group_check=True,
                            )
                        (ja, offa, wa, qoa), (jb, offb, wb, qob) = specs
                        if p == 0:
                            nc.vector.tensor_copy(
                                acc[:, qoa: qoa + wa], et2[:, offa: offa + wa]
                            )
                            nc.vector.tensor_add(
                                acc[:, qob: qob + wb], acc[:, qob: qob + wb],
                                et2[:, offb: offb + wb],
                            )
                        else:
                            acc_eng.tensor_add(
                                acc[:, qoa: qoa + wa], acc[:, qoa: qoa + wa],
                                et2[:, offa: offa + wa],
                            )
                            acc_eng.tensor_add(
                                acc[:, qob: qob + wb], acc[:, qob: qob + wb],
                                et2[:, offb: offb + wb],
                            )

                    prev = None
                    for p in range(2 * ci + 2):
                        ja, jb = 2 * p, 2 * p + 1
                        q0a = max(ja * 128, 512 * ci)
                        wa = 512 * ci + 512 - q0a
                        q0b = max(jb * 128, 512 * ci)
                        wb = 512 * ci + 512 - q0b
                        qoa, qob = q0a - 512 * ci, q0b - 512 * ci
                        pp = ps.tile([128, 1024], F32, tag="s2", bufs=2,
                                      name=f"s{b}{h}{ci}_{p}")
                        nc.tensor.matmul(
                            pp[:, 0:wa],
                            kT_sb[:, (b * 16 + ja) * 128: (b * 16 + ja) * 128 + 128],
                            q_sb[:, qbase + q0a: qbase + q0a + wa],
                            start=True, stop=True, skip_group_check=True,
                        )
                        # region b needs its own has_written clear only when
                        # it lands in bank 1; when it shares bank 0 with
                        # region a (wa < 512), region a's start already
                        # cleared its bits and a second clear is not needed.
                        nc.tensor.matmul(
                            pp[:, wa: wa + wb],
                            kT_sb[:, (b * 16 + jb) * 128: (b * 16 + jb) * 128 + 128],
                            q_sb[:, qbase + q0b: qbase + q0b + wb],
                            start=(wa == 512), stop=True, skip_group_check=True,
                        )
                        if ja >= 4 * ci:
                            nc.vector.tensor_add(
                                pp[:, 0:128], pp[:, 0:128], trim_sb[:]
                            )
                        if jb >= 4 * ci:
                            nc.vector.tensor_add(
                                pp[:, wa: wa + 128], pp[:, wa: wa + 128],
                                trim_sb[:],
                            )
                        et2 = es.tile([128, 1024], BF, tag="e", bufs=6,
                                      name=f"e{b}{h}{ci}_{p}")
                        nc.scalar.activation(
                            et2[:, 0: wa + wb], pp[:, 0: wa + wb],
                            mybir.ActivationFunctionType.Exp, scale=SCALE,
                        )
                        if prev is not None:
                            pv_and_acc(*prev)
                        prev = (p, et2,
                                ((ja, 0, wa, qoa), (jb, wa, wb, qob)))
                        yield
                    pv_and_acc(*prev)

                    # tail: ones-stationary rowsum matmul -> [1,512]
                    # denominators in PSUM, [1,512] DVE reciprocal, then the
                    # ones^T (x) recip broadcast matmul and one normalizing
                    # multiply.
                    psr = ps.tile([1, 512], F32, tag="aux", bufs=1,
                                   padded_shape=[128, 512],
                                   name=f"rs{b}{h}{ci}")
                    nc.tensor.matmul(
                        psr[0:1, :], onec_sb[:], acc[:],
                        start=True, stop=True, skip_group_check=True,
                    )
                    rc_row = rns.tile([1, 512], BF, tag="rcrow", bufs=2)
                    with nc.allow_low_precision(
                        reason="softmax reciprocal consumed as bf16 matmul "
                               "operand; matches baseline precision"
                    ):
                        nc.vector.reciprocal(rc_row[:], psr[:])
                    bc_ps = ps.tile([128, 512], F32, tag="aux", bufs=1,
                                     name=f"bc{b}{h}{ci}")
                    nc.tensor.matmul(
                        bc_ps[:], oner_sb[:], rc_row[:],
                        start=True, stop=True, skip_group_check=True,
                    )
                    bc = rns.tile([128, 512], F32, tag="bc", bufs=2)
                    nc.vector.tensor_copy(bc[:], bc_ps[:])
                    an = rns.tile([128, 512], BF, tag="an", bufs=6)
                    nc.vector.tensor_mul(an[:], ps_attn[:], bc[:])
                    nc.sync.dma_start(
                        a2a_in[h][128 * (b * 4 + ci): 128 * (b * 4 + ci) + 128, :],
                        an[:],
                    )
                    yield

                # continuous 2-in-flight worklist; fire each head's AllToAll
                # the moment its last chain drains
                todo = [(b, h, ci)
                        for h in range(4) for b in range(B)
                        for ci in (0, 3, 1, 2)]
                todo.reverse()
                left = {h: 2 * 4 for h in range(4)}
                active = [[todo[-1][1], attn_chain(*todo.pop())],
                          [todo[-1][1], attn_chain(*todo.pop())],
                          [todo[-1][1], attn_chain(*todo.pop())]]
                while active:
                    for ent in list(active):
                        if next(ent[1], StopIteration) is StopIteration:
                            active.remove(ent)
                            left[ent[0]] -= 1
                            if left[ent[0]] == 0:
                                head_done(ent[0])
                            if todo:
                                active.append(
                                    [todo[-1][1], attn_chain(*todo.pop())]
                                )

            # ======== phase D: output projection for this core's rows ========
            # Two 4-bank PSUM sets; each cg's k=6,7 head-groups (the heads
            # whose AllToAll lands last) are deferred until after the next
            # cg's k=0..5, so the final AllToAll is fully hidden.
            if True:
                wtq = [None] * 8  # per-cg {ht: wt slice}

                def d_load_wt(cg, k):
                    wq4 = ws.tile([128, 2048], BF, tag="wo",
                                  bufs=16, name=f"wt{cg}_{k}")
                    i0, lv = 4 * (k % 2), k // 2
                    src_ap = woT[:].rearrange(
                        "(a l p) c -> p a l c", p=128, l=4
                    )[:, i0: i0 + 4, lv,
                      cg * 512: (cg + 1) * 512]
                    engs3[k % 3].dma_start(
                        wq4[:].rearrange("p (b c) -> p b c", b=4),
                        src_ap,
                    )
                    for n, ht2 in enumerate(ht_order[4 * k: 4 * k + 4]):
                        wtq[cg][ht2] = wq4[:, n * 512: (n + 1) * 512]

                po_sets = [None] * 8

                def d_alloc(cg):
                    if cg % 2 == 0:
                        t = [ps.tile([128, 512], F32, tag="pa", bufs=3,
                                     name=f"po{cg}_{i}")[:] for i in range(3)]
                        t.append(ps.tile([128, 512], F32, tag="aux", bufs=1,
                                         name=f"po{cg}_3")[:])
                        return t
                    s0 = ps.tile([128, 1024], F32, tag="s2", bufs=2,
                                 name=f"po{cg}_01")
                    s1 = ps.tile([128, 1024], F32, tag="s2", bufs=2,
                                 name=f"po{cg}_23")
                    return [s0[:, 0:512], s0[:, 512:1024],
                            s1[:, 0:512], s1[:, 512:1024]]

                def d_emit(cg, ks):
                    po = po_sets[cg]
                    for k in ks:
                        if cg == 0:
                            wt4 = [wt0[ht] for ht in ht_order[4 * k: 4 * k + 4]]
                        else:
                            d_load_wt(cg, k)
                            wt4 = [wtq[cg][ht] for ht in ht_order[4 * k: 4 * k + 4]]
                        for n, ht in enumerate(ht_order[4 * k: 4 * k + 4]):
                            n_ht = 4 * k + n
                            for rt in range(4):
                                nc.tensor.matmul(
                                    po[rt],
                                    at_sb[:, ht * 512 + rt * 128:
                                          ht * 512 + (rt + 1) * 128],
                                    wt4[n],
                                    start=(n_ht == 0), stop=(n_ht == 31),
                                    skip_group_check=True,
                                )

                def d_copyout(cg):
                    for rt in range(4):
                        ot = osp.tile([128, 512], F32, tag="o")
                        nc.vector.tensor_copy(ot[:], po_sets[cg][rt])
                        engs3[rt % 2].dma_start(
                            out[rt * 128: (rt + 1) * 128,
                                cg * 512: (cg + 1) * 512],
                            ot[:],
                        )
                    po_sets[cg] = None

                for cg in range(9):
                    if cg < 8:
                        wtq[cg] = {}
                        po_sets[cg] = d_alloc(cg)
                        d_emit(cg, range(6))
                    if cg >= 1:
                        d_emit(cg - 1, range(6, 8))
                        d_copyout(cg - 1)


def _build():
    nc = bacc.Bacc("TRN2", target_bir_lowering=False, debug=False, num_devices=NC)
    xT = nc.dram_tensor("xT", [D, BS], BF, kind="ExternalInput")
    wqkvT = nc.dram_tensor("wqkvT", [D, 768], BF, kind="ExternalInput")
    woT = nc.dram_tensor("woT", [D, D], BF, kind="ExternalInput")
    ccR = nc.dram_tensor("ccR", [128, NRB * 256], BF, kind="ExternalInput")
    ssR = nc.dram_tensor("ssR", [128, NRB * 256], BF, kind="ExternalInput")
    trim = nc.dram_tensor("trim", [128, 128], F32, kind="ExternalInput")
    iden = nc.dram_tensor("iden", [128, 128], BF, kind="ExternalInput")
    out = nc.dram_tensor("out", [R, D], F32, kind="ExternalOutput")
    with tile.TileContext(nc) as tc:
        _emit(nc, tc, (xT, wqkvT, woT, ccR, ssR, trim, iden, out))
    nc.compile()
    return nc


_NC = None


def kernel(x, wq, wk, wv, wo, freqs_cos, freqs_sin, mask, start_pos):
    global _NC
    if _NC is None:
        _NC = _build()
    nc = _NC
    bf = ml_dtypes.bfloat16

    x = np.asarray(x, dtype=np.float32)
    xT = np.ascontiguousarray(x.reshape(BS, D).T).astype(bf)

    perm = np.concatenate([np.arange(0, HD, 2), np.arange(1, HD, 2)])
    wqTp = np.asarray(wq, np.float32).T.reshape(D, H, HD)[:, :, perm]
    wkTp = np.asarray(wk, np.float32).T.reshape(D, HKV, HD)[:, :, perm]
    wvT = np.asarray(wv, np.float32).T.reshape(D, HKV, HD)
    woT = np.ascontiguousarray(np.asarray(wo, np.float32).T).astype(bf)

    fc = np.asarray(freqs_cos, np.float32)
    fs = np.asarray(freqs_sin, np.float32)
    # row-major RoPE tables per row block, replicated x4 along free axis
    pos = (np.arange(BS) % S).reshape(NRB, 128)
    ccR = np.tile(fc[pos], (1, 1, 4)).transpose(1, 0, 2).reshape(128, NRB * 256)
    ssR = np.tile(fs[pos], (1, 1, 4)).transpose(1, 0, 2).reshape(128, NRB * 256)
    ccR = np.ascontiguousarray(ccR).astype(bf)
    ssR = np.ascontiguousarray(ssR).astype(bf)

    trim = np.where(
        np.arange(128)[:, None] > np.arange(128)[None, :], -1e30, 0.0
    ).astype(np.float32)
    iden = np.eye(128, dtype=bf)

    in_maps = []
    for c in range(NC):
        wqkv = np.concatenate(
            [
                wqTp[:, 4 * c: 4 * c + 4].reshape(D, 512),
                wkTp[:, c],
                wvT[:, c],
            ],
            axis=1,
        ).astype(bf)
        in_maps.append(
            {
                "xT": xT,
                "wqkvT": np.ascontiguousarray(wqkv),
                "woT": woT,
                "ccR": ccR,
                "ssR": ssR,
                "trim": trim,
                "iden": iden,
            }
        )

    res = bass_utils.run_bass_kernel_spmd(
        nc, in_maps, core_ids=list(range(NC)), trace=PROFILE, tmpdir=TMPDIR
    )
    if PROFILE:
        print(f"HW exec time: {res.exec_time_ns} ns")
        if res.instructions_and_trace is not None:
            print(f"trace: {res.instructions_and_trace[1]}")

    out_full = np.empty((BS, D), dtype=np.float32)
    for c in range(NC):
        out_full[R * c: R * (c + 1)] = res.results[c]["out"]
    return out_full.reshape(B, S, D)


# revision 10
# speedup vs baseline: 1.1433x; 1.0463x over previous
"""Distributed Trainium2 attention kernel (8 NeuronCores).

Strategy: tensor-parallel over heads for QKV projection + attention
(4 query heads + their 1 shared KV head per core), then an AllToAll
switches to row-sharding so each core computes the output projection for
its 512 rows with the full wo. Host reassembles rows. All matmuls run in
bf16 with fp32 PSUM accumulation.

The PE sustains ~0.5 ns/moving-column (2.0 GHz effective under the
chip-wide power state) with LDWEIGHTS and semaphore updates fully hidden,
so the design minimizes *streamed columns* and keeps every other engine
off the PE's critical path:

- Phase B (QKV+RoPE): x tiles stationary, wqkv moving; all of x-group
  0 and 1 plus the weights are DMA'd up front (the JIT issue scheme
  starved the PE for ~30us at startup via engine-FIFO head-of-line
  blocking).
- Phase C (attention, [keys, q] layout): score tiles for consecutive
  key-block PAIRS are packed contiguously into one 2-bank PSUM tile so
  ONE exp instruction covers both (halves ScalarE's ~300ns/op fixed
  cost; ScalarE is the phase C bottleneck). The softmax denominator is
  DVE/GpSimd-accumulated from the exp'd tiles (whole chains alternate
  engines; the adds of one chain are serial anyway). The chain tail is
  one ones-stationary rowsum matmul ([1,512] = denominators), a [1,512]
  DVE reciprocal, and the ones^T (x) recip broadcast matmul -- no PE
  transposes and no ScalarE involvement.
- Phase D (output projection): two 4-bank PSUM sets; each cg's last
  head-group (the heads whose AllToAll lands last) is deferred until
  after the NEXT cg's first 24 head-tiles are emitted, so the PE always
  has ~25us of runway while the final AllToAll is in flight.

RoPE is applied in row-major layout via a host-side even/odd column
permutation of wq/wk (rotation becomes contiguous half-block arithmetic),
then q/k are transposed to [head_dim, rows] on the TensorEngine.
"""

import numpy as np
import ml_dtypes

import concourse.bass as bass
import concourse.mybir as mybir
import concourse.tile as tile
from concourse import bacc
from concourse import bass_utils

B, S, D = 2, 2048, 4096
H, HKV, HD = 32, 8, 128
HD2 = HD // 2
NC = 8
HL = H // NC            # 4 local q heads per core
BS = B * S              # 4096 global rows
R = BS // NC            # 512 output rows per core
NRB = BS // 128         # 32 row blocks
NDT = D // 128          # 32 contraction tiles
SCALE = 1.0 / float(np.sqrt(HD))
BF = mybir.dt.bfloat16
F32 = mybir.dt.float32

PROFILE = False         # set by test.py for neuron-profile capture
TMPDIR = None           # set by test.py to keep the trace dir


def _emit(nc, tc, io):
    xT, wqkvT, woT, ccR, ssR, trim, iden, out = io

    engs3 = (nc.sync, nc.scalar, nc.gpsimd)

    with (
        tc.tile_pool(name="cbuf", bufs=1) as cbuf,
        tc.tile_pool(name="qbuf", bufs=1) as qbuf,
        tc.tile_pool(name="kvbuf", bufs=1) as kvbuf,
        tc.tile_pool(name="dram", bufs=1, space="DRAM") as dram,
        tc.tile_pool(name="ps", bufs=1, space="PSUM") as ps,
    ):
        # ---- long-lived SBUF state ----
        q_sb = qbuf.tile([128, HL * BS], BF, tag="q")     # col = h*4096 + row
        kT_sb = kvbuf.tile([128, BS], BF, tag="k")        # col = row
        v_sb = kvbuf.tile([128, BS], BF, tag="v")         # col = rb*128 + hd

        trim_sb = cbuf.tile([128, 128], F32, tag="tm")
        iden_sb = cbuf.tile([128, 128], BF, tag="idn")
        onec_sb = cbuf.tile([128, 1], BF, tag="onec")
        oner_sb = cbuf.tile([1, 128], BF, tag="oner")

        # one AllToAll per local head (fired as soon as that head's chains
        # drain) so phase D's inputs arrive progressively
        a2a_in = [dram.tile([BS // 4, R], BF, name=f"a2a_in{h}") for h in range(4)]
        a2a_out = [dram.tile([BS // 4, R], BF, name=f"a2a_out{h}") for h in range(4)]

        # ================= phase B: QKV projection + RoPE =================
        with (
            tc.tile_pool(name="wbuf", bufs=1) as wbuf,
            tc.tile_pool(name="xs", bufs=1) as xs,
            tc.tile_pool(name="cs", bufs=6) as cs,
            tc.tile_pool(name="ts", bufs=8) as ts,
        ):
            # resident QKV weights: col = dt*768 + [0:512 q | 512:640 k | 640:768 v]
            w_sb = wbuf.tile([128, NDT * 768], BF, tag="w")
            # x tiles: one [128, 4*512] quad covers 4 d-slices x 512 rows
            xg = [[None] * (NDT // 4) for _ in range(8)]

            def issue_xg(g, dq):
                t = xs.tile([128, 2048], BF, tag="x", bufs=16, name=f"x{g}_{dq}")
                src_ap = xT[dq * 512:(dq + 1) * 512, g * 512:(g + 1) * 512] \
                    .rearrange("(b p) c -> p b c", p=128)
                dst_ap = t[:].rearrange("p (b c) -> p b c", b=4)
                engs3[(g * 8 + dq + 1) % 3].dma_start(dst_ap, src_ap)
                xg[g][dq] = t

            def issue_w(dt):
                engs3[dt % 3].dma_start(
                    w_sb[:, dt * 768: dt * 768 + 768],
                    wqkvT[dt * 128: (dt + 1) * 128, :],
                )

            nc.sync.dma_start(trim_sb[:], trim[:])
            nc.scalar.dma_start(iden_sb[:], iden[:])
            nc.vector.memset(onec_sb[:], 1.0)
            nc.vector.memset(oner_sb[:], 1.0)
            # Everything the first two row-groups need is issued up front
            # with no buffer-recycle waits at the head of any engine FIFO:
            # first 4 weight tiles, then x-group 0 interleaved with the
            # remaining weights, then x-group 1 (xs has exactly 16 slots).
            for dt in range(4):
                issue_w(dt)
            issue_xg(0, 0)
            nxt = 4
            for dq in range(1, 8):
                issue_xg(0, dq)
                for _ in range(3):
                    if nxt < NDT:
                        issue_w(nxt)
                        nxt += 1
            while nxt < NDT:
                issue_w(nxt)
                nxt += 1

            # rope tables: one [128, 1024] tile covers 4 row blocks
            csq = {}

            def issue_cs(q):
                cct = cs.tile([128, 1024], BF, tag="cc", bufs=3, name=f"cc{q}")
                engs3[q % 3].dma_start(cct[:], ccR[:, q * 1024: (q + 1) * 1024])
                sst = cs.tile([128, 1024], BF, tag="ss", bufs=3, name=f"ss{q}")
                engs3[(q + 1) % 3].dma_start(sst[:], ssR[:, q * 1024: (q + 1) * 1024])
                csq[q] = (cct, sst)

            issue_cs(0)
            for dq in range(8):
                issue_xg(1, dq)

            # rope tails are emitted one rb late, behind rb+1's matmuls
            def b_rope_tail_q(rb, ps_q):
                if rb % 4 == 0 and rb // 4 + 1 < 8:
                    issue_cs(rb // 4 + 1)
                cq, sq = csq[rb // 4]
                cct = cq[:, (rb % 4) * 256: (rb % 4 + 1) * 256]
                sst = sq[:, (rb % 4) * 256: (rb % 4 + 1) * 256]

                qe = ps_q[:].rearrange("p (h d) -> p h d", d=128)[:, :, 0:HD2]
                qo = ps_q[:].rearrange("p (h d) -> p h d", d=128)[:, :, HD2:HD]
                t1 = ts.tile([128, 256], BF, tag="t")
                t2 = ts.tile([128, 256], BF, tag="t")
                t3 = ts.tile([128, 256], BF, tag="t")
                t4 = ts.tile([128, 256], BF, tag="t")
                nc.vector.tensor_mul(t1[:], qe, cct)
                nc.vector.tensor_mul(t2[:], qo, sst)
                nc.vector.tensor_mul(t3[:], qe, sst)
                nc.vector.tensor_mul(t4[:], qo, cct)
                qrot = ts.tile([128, 512], BF, tag="qr")
                qre = qrot[:].rearrange("p (h d) -> p h d", d=128)[:, :, 0:HD2]
                qro = qrot[:].rearrange("p (h d) -> p h d", d=128)[:, :, HD2:HD]
                nc.vector.tensor_sub(qre, t1[:], t2[:])
                nc.vector.tensor_add(qro, t3[:], t4[:])
                return (qrot, cct, sst)

            def b_transpose_tail_q(rb, qrot):
                ps_tq = ps.tile([128, 512], BF, tag="aux", bufs=1, padded_shape=[128, 1024])
                for h in range(HL):
                    nc.tensor.transpose(
                        ps_tq[:, h * 128: (h + 1) * 128],
                        qrot[:, h * 128: (h + 1) * 128],
                        iden_sb[:],
                    )
                q_dst = (
                    q_sb[:]
                    .rearrange("p (h r) -> p h r", h=HL)
                    [:, :, rb * 128: (rb + 1) * 128]
                )
                nc.vector.tensor_copy(
                    q_dst, ps_tq[:].rearrange("p (h r) -> p h r", h=HL)
                )

            def b_rope_tail_kv(rb, ps_kv, cct, sst):
                ke = ps_kv[:, 0:HD2]
                ko = ps_kv[:, HD2:HD]
                u1 = ts.tile([128, 64], BF, tag="u")
                u2 = ts.tile([128, 64], BF, tag="u")
                u3 = ts.tile([128, 64], BF, tag="u")
                u4 = ts.tile([128, 64], BF, tag="u")
                nc.vector.tensor_mul(u1[:], ke, cct[:, 0:HD2])
                nc.vector.tensor_mul(u2[:], ko, sst[:, 0:HD2])
                nc.vector.tensor_mul(u3[:], ke, sst[:, 0:HD2])
                nc.vector.tensor_mul(u4[:], ko, cct[:, 0:HD2])

                krot = ts.tile([128, 128], BF, tag="kr")
                nc.vector.tensor_sub(krot[:, 0:HD2], u1[:], u2[:])
                nc.vector.tensor_add(krot[:, HD2:HD], u3[:], u4[:])

                # v: plain copy to row-major storage
                nc.scalar.activation(
                    v_sb[:, rb * 128: (rb + 1) * 128], ps_kv[:, 128:256],
                    mybir.ActivationFunctionType.Copy,
                )
                return (krot,)

            def b_transpose_tail_kv(rb, krot):
                ps_tk = ps.tile([128, 128], BF, tag="aux", bufs=1, padded_shape=[128, 1024])
                nc.tensor.transpose(ps_tk[:], krot[:], iden_sb[:])
                nc.vector.tensor_copy(kT_sb[:, rb * 128: (rb + 1) * 128], ps_tk[:])

            pending = None
            rot = None
            for rb in range(NRB):
                g, ri = rb // 4, rb % 4
                ps_q = ps.tile([128, 512], F32, tag="pa", bufs=3)
                ps_kv = ps.tile([128, 256], F32, tag="s2", bufs=2, padded_shape=[128, 1024])
                for dt in range(NDT):
                    xt = xg[g][dt // 4][:, (dt % 4) * 512 + ri * 128:
                                        (dt % 4) * 512 + (ri + 1) * 128]
                    st, sp = dt == 0, dt == NDT - 1
                    nc.tensor.matmul(
                        ps_q[:], xt, w_sb[:, dt * 768: dt * 768 + 512],
                        start=st, stop=sp,
                    )
                    nc.tensor.matmul(
                        ps_kv[:], xt, w_sb[:, dt * 768 + 512: dt * 768 + 768],
                        start=st, stop=sp,
                    )
                    # prefetch next row-group's x quads, spread over this group
                    # (groups 0 and 1 were fully issued up front)
                    if ri == 2 and g >= 1 and g + 1 < 8 and dt % 4 == 1:
                        issue_xg(g + 1, dt // 4)
                    if dt == 2 and pending is not None:
                        pq = b_rope_tail_q(pending[0], pending[1])
                        pkv = b_rope_tail_kv(pending[0], pending[2], pq[1], pq[2])
                        rot = (pending[0], pq[0]) + pkv
                        pending = None
                    if dt == 12 and rot is not None:
                        b_transpose_tail_q(rot[0], rot[1])
                        b_transpose_tail_kv(rot[0], rot[2])
                        rot = None
                pending = (rb, ps_q, ps_kv)
            pq = b_rope_tail_q(pending[0], pending[1])
            pkv = b_rope_tail_kv(pending[0], pending[2], pq[1], pq[2])
            b_transpose_tail_q(pending[0], pq[0])
            b_transpose_tail_kv(pending[0], pkv[0])

        # ============ phase C: causal attention (flipped PV) ============
        with (
            tc.tile_pool(name="es", bufs=6) as es,
            tc.tile_pool(name="rns", bufs=4) as rns,
            tc.tile_pool(name="abuf", bufs=1) as abuf,
            tc.tile_pool(name="ws", bufs=1) as ws,
            tc.tile_pool(name="osp", bufs=4) as osp,
        ):
            at_sb = abuf.tile([128, 32 * 512], BF, tag="at")  # col = ht*512+row
            # head-major so the earliest AllToAlls feed phase D's first
            # accumulation steps
            ht_order = [4 * i + l for l in range(4) for i in range(8)]
            wt0 = {}  # prefetched wo tiles for cg 0

            if True:

                def head_done(h):
                    """Fire head h's AllToAll + phase-D prefetches."""
                    nc.gpsimd.collective_compute(
                        "AllToAll",
                        mybir.AluOpType.bypass,
                        replica_groups=[list(range(NC))],
                        ins=[a2a_in[h].opt()],
                        outs=[a2a_out[h].opt()],
                    )
                    dst_ap = at_sb[:].rearrange(
                        "p (i c) -> p i c", c=512
                    )[:, h::4, :]
                    src_ap = a2a_out[h][:].rearrange("(i p) c -> p i c", p=128)
                    nc.gpsimd.dma_start(dst_ap, src_ap)
                    if h == 0:
                        for k in range(0, 32, 4):
                            wt = ws.tile([128, 2048], BF, tag="wo", bufs=16,
                                         name=f"wt0_{k}")
                            i0, lv = k % 8, k // 8
                            src_ap = woT[:].rearrange(
                                "(a l p) c -> p a l c", p=128, l=4
                            )[:, i0: i0 + 4, lv, 0:512]
                            nc.gpsimd.dma_start(
                                wt[:].rearrange("p (b c) -> p b c", b=4), src_ap
                            )
                            for n, ht in enumerate(ht_order[k: k + 4]):
                                wt0[ht] = wt[:, n * 512: (n + 1) * 512]

                def attn_chain(b, h, ci):
                    # pair-packed j pipeline: two key blocks' score tiles land
                    # contiguously in one 2-bank PSUM tile -> one exp, then PV
                    # (v-stationary, [hd, q] PSUM accumulation) with the
                    # softmax denominator accumulated from the exp'd tiles
                    # (vector/gpsimd alternating by chain).
                    qbase = h * BS + b * S
                    jmax = 4 * ci + 3
                    # gpsimd tensor_add is ~2.6x slower than the DVE and its
                    # queue blocks ~13us at each head boundary (collective +
                    # at-copy head the FIFO), so it only takes the largest
                    # late-in-head chains (~20% of the add volume)
                    acc_eng = nc.gpsimd if (b == 1 and ci == 3) else nc.vector
                    ps_attn = ps.tile([128, 512], F32, tag="pa", bufs=3,
                                       name=f"pa{b}{h}{ci}")
                    acc = rns.tile([128, 512], BF, tag="acc", bufs=3,
                                   name=f"acc{b}{h}{ci}")

                    def pv_and_acc(p, et2, specs):
                        for (j, off, w, qo) in specs:
                            kcol = (b * 16 + j) * 128
                            nc.tensor.matmul(
                                ps_attn[:, qo: qo + w],
                                v_sb[:, kcol: kcol + 128],
                                et2[:, off: off + w],
                                start=(j == 0), stop=(j == jmax),
                                skip_group_check=True,
                            )
                        (ja, offa, wa, qoa), (jb, offb, wb, qob) = specs
                        if p == 0:
                            nc.vector.tensor_copy(
                                acc[:, qoa: qoa + wa], et2[:, offa: offa + wa]
                            )
                            nc.vector.tensor_add(
                                acc[:, qob: qob + wb], acc[:, qob: qob + wb],
                                et2[:, offb: offb + wb],
                            )
                        else:
                            acc_eng.tensor_add(
                                acc[:, qoa: qoa + wa], acc[:, qoa: qoa + wa],
                                et2[:, offa: offa + wa],
                            )
                            acc_eng.tensor_add(
                                acc[:, qob: qob + wb], acc[:, qob: qob + wb],
                                et2[:, offb: offb + wb],
                            )

                    prev = None
                    for p in range(2 * ci + 2):
                        ja, jb = 2 * p, 2 * p + 1
                        q0a = max(ja * 128, 512 * ci)
                        wa = 512 * ci + 512 - q0a
                        q0b = max(jb * 128, 512 * ci)
                        wb = 512 * ci + 512 - q0b
                        qoa, qob = q0a - 512 * ci, q0b - 512 * ci
                        pp = ps.tile([128, 1024], F32, tag="s2", bufs=2,
                                      name=f"s{b}{h}{ci}_{p}")
                        nc.tensor.matmul(
                            pp[:, 0:wa],
                            kT_sb[:, (b * 16 + ja) * 128: (b * 16 + ja) * 128 + 128],
                            q_sb[:, qbase + q0a: qbase + q0a + wa],
                            start=True, stop=True, skip_group_check=True,
                        )
                        # region b needs its own has_written clear only when
                        # it lands in bank 1; when it shares bank 0 with
                        # region a (wa < 512), region a's start already
                        # cleared its bits and a second clear is not needed.
                        nc.tensor.matmul(
                            pp[:, wa: wa + wb],
                            kT_sb[:, (b * 16 + jb) * 128: (b * 16 + jb) * 128 + 128],
                            q_sb[:, qbase + q0b: qbase + q0b + wb],
                            start=(wa == 512), stop=True, skip_group_check=True,
                        )
                        if ja >= 4 * ci:
                            nc.vector.tensor_add(
                                pp[:, 0:128], pp[:, 0:128], trim_sb[:]
                            )
                        if jb >= 4 * ci:
                            nc.vector.tensor_add(
                                pp[:, wa: wa + 128], pp[:, wa: wa + 128],
                                trim_sb[:],
                            )
                        et2 = es.tile([128, 1024], BF, tag="e", bufs=6,
                                      name=f"e{b}{h}{ci}_{p}")
                        nc.scalar.activation(
                            et2[:, 0: wa + wb], pp[:, 0: wa + wb],
                            mybir.ActivationFunctionType.Exp, scale=SCALE,
                        )
                        if prev is not None:
                            pv_and_acc(*prev)
                        prev = (p, et2,
                                ((ja, 0, wa, qoa), (jb, wa, wb, qob)))
                        yield
                    pv_and_acc(*prev)

                    # tail: ones-stationary rowsum matmul -> [1,512]
                    # denominators in PSUM, [1,512] DVE reciprocal, then the
                    # ones^T (x) recip broadcast matmul and one normalizing
                    # multiply.
                    psr = ps.tile([1, 512], F32, tag="aux", bufs=1,
                                   padded_shape=[128, 512],
                                   name=f"rs{b}{h}{ci}")
                    nc.tensor.matmul(
                        psr[0:1, :], onec_sb[:], acc[:],
                        start=True, stop=True, skip_group_check=True,
                    )
                    rc_row = rns.tile([1, 512], BF, tag="rcrow", bufs=2)
                    with nc.allow_low_precision(
                        reason="softmax reciprocal consumed as bf16 matmul "
                               "operand; matches baseline precision"
                    ):
                        nc.vector.reciprocal(rc_row[:], psr[:])
                    bc_ps = ps.tile([128, 512], F32, tag="aux", bufs=1,
                                     name=f"bc{b}{h}{ci}")
                    nc.tensor.matmul(
                        bc_ps[:], oner_sb[:], rc_row[:],
                        start=True, stop=True, skip_group_check=True,
                    )
                    bc = rns.tile([128, 512], F32, tag="bc", bufs=2)
                    nc.vector.tensor_copy(bc[:], bc_ps[:])
                    an = rns.tile([128, 512], BF, tag="an", bufs=6)
                    nc.vector.tensor_mul(an[:], ps_attn[:], bc[:])
                    nc.sync.dma_start(
                        a2a_in[h][128 * (b * 4 + ci): 128 * (b * 4 + ci) + 128, :],
                        an[:],
                    )
                    yield

                # continuous 2-in-flight worklist; fire each head's AllToAll
                # the moment its last chain drains
                todo = [(b, h, ci)
                        for h in range(4) for b in range(B)
                        for ci in (0, 3, 1, 2)]
                todo.reverse()
                left = {h: 2 * 4 for h in range(4)}
                active = [[todo[-1][1], attn_chain(*todo.pop())],
                          [todo[-1][1], attn_chain(*todo.pop())],
                          [todo[-1][1], attn_chain(*todo.pop())]]
                while active:
                    for ent in list(active):
                        if next(ent[1], StopIteration) is StopIteration:
                            active.remove(ent)
                            left[ent[0]] -= 1
                            if left[ent[0]] == 0:
                                head_done(ent[0])
                            if todo:
                                active.append(
                                    [todo[-1][1], attn_chain(*todo.pop())]
                                )

            # ======== phase D: output projection for this core's rows ========
            # Two 4-bank PSUM sets; each cg's k=6,7 head-groups (the heads
            # whose AllToAll lands last) are deferred until after the next
            # cg's k=0..5, so the final AllToAll is fully hidden.
            if True:
                wtq = [None] * 8  # per-cg {ht: wt slice}

                def d_load_wt(cg, k):
                    wq4 = ws.tile([128, 2048], BF, tag="wo",
                                  bufs=16, name=f"wt{cg}_{k}")
                    i0, lv = 4 * (k % 2), k // 2
                    src_ap = woT[:].rearrange(
                        "(a l p) c -> p a l c", p=128, l=4
                    )[:, i0: i0 + 4, lv,
                      cg * 512: (cg + 1) * 512]
                    engs3[k % 3].dma_start(
                        wq4[:].rearrange("p (b c) -> p b c", b=4),
                        src_ap,
                    )
                    for n, ht2 in enumerate(ht_order[4 * k: 4 * k + 4]):
                        wtq[cg][ht2] = wq4[:, n * 512: (n + 1) * 512]

                po_sets = [None] * 8

                def d_alloc(cg):
                    if cg % 2 == 0:
                        t = [ps.tile([128, 512], F32, tag="pa", bufs=3,
                                     name=f"po{cg}_{i}")[:] for i in range(3)]
                        t.append(ps.tile([128, 512], F32, tag="aux", bufs=1,
                                         name=f"po{cg}_3")[:])
                        return t
                    s0 = ps.tile([128, 1024], F32, tag="s2", bufs=2,
                                 name=f"po{cg}_01")
                    s1 = ps.tile([128, 1024], F32, tag="s2", bufs=2,
                                 name=f"po{cg}_23")
                    return [s0[:, 0:512], s0[:, 512:1024],
                            s1[:, 0:512], s1[:, 512:1024]]

                def d_emit(cg, ks):
                    po = po_sets[cg]
                    for k in ks:
                        if cg == 0:
                            wt4 = [wt0[ht] for ht in ht_order[4 * k: 4 * k + 4]]
                        else:
                            d_load_wt(cg, k)
                            wt4 = [wtq[cg][ht] for ht in ht_order[4 * k: 4 * k + 4]]
                        for n, ht in enumerate(ht_order[4 * k: 4 * k + 4]):
                            n_ht = 4 * k + n
                            for rt in range(4):
                                nc.tensor.matmul(
                                    po[rt],
                                    at_sb[:, ht * 512 + rt * 128:
                                          ht * 512 + (rt + 1) * 128],
                                    wt4[n],
                                    start=(n_ht == 0), stop=(n_ht == 31),
                                    skip_group_check=True,
                                )

                def d_copyout(cg):
                    for rt in range(4):
                        ot = osp.tile([128, 512], F32, tag="o")
                        nc.vector.tensor_copy(ot[:], po_sets[cg][rt])
                        engs3[rt % 2].dma_start(
                            out[rt * 128: (rt + 1) * 128,
                                cg * 512: (cg + 1) * 512],
                            ot[:],
                        )
                    po_sets[cg] = None

                for cg in range(9):
                    if cg < 8:
                        wtq[cg] = {}
                        po_sets[cg] = d_alloc(cg)
                        d_emit(cg, range(6))
                    if cg >= 1:
                        d_emit(cg - 1, range(6, 8))
                        d_copyout(cg - 1)


def _build():
    nc = bacc.Bacc("TRN2", target_bir_lowering=False, debug=False, num_devices=NC)
    xT = nc.dram_tensor("xT", [D, BS], BF, kind="ExternalInput")
    wqkvT = nc.dram_tensor("wqkvT", [D, 768], BF, kind="ExternalInput")
    woT = nc.dram_tensor("woT", [D, D], BF, kind="ExternalInput")
    ccR = nc.dram_tensor("ccR", [128, NRB * 256], BF, kind="ExternalInput")
    ssR = nc.dram_tensor("ssR", [128, NRB * 256], BF, kind="ExternalInput")
    trim = nc.dram_tensor("trim", [128, 128], F32, kind="ExternalInput")
    iden = nc.dram_tensor("iden", [128, 128], BF, kind="ExternalInput")
    out = nc.dram_tensor("out", [R, D], F32, kind="ExternalOutput")
    with tile.TileContext(nc) as tc:
        _emit(nc, tc, (xT, wqkvT, woT, ccR, ssR, trim, iden, out))
    nc.compile()
    return nc


_NC = None


def kernel(x, wq, wk, wv, wo, freqs_cos, freqs_sin, mask, start_pos):
    global _NC
    if _NC is None:
        _NC = _build()
    nc = _NC
    bf = ml_dtypes.bfloat16

    x = np.asarray(x, dtype=np.float32)
    xT = np.ascontiguousarray(x.reshape(BS, D).T).astype(bf)

    perm = np.concatenate([np.arange(0, HD, 2), np.arange(1, HD, 2)])
    wqTp = np.asarray(wq, np.float32).T.reshape(D, H, HD)[:, :, perm]
    wkTp = np.asarray(wk, np.float32).T.reshape(D, HKV, HD)[:, :, perm]
    wvT = np.asarray(wv, np.float32).T.reshape(D, HKV, HD)
    woT = np.ascontiguousarray(np.asarray(wo, np.float32).T).astype(bf)

    fc = np.asarray(freqs_cos, np.float32)
    fs = np.asarray(freqs_sin, np.float32)
    # row-major RoPE tables per row block, replicated x4 along free axis
    pos = (np.arange(BS) % S).reshape(NRB, 128)
    ccR = np.tile(fc[pos], (1, 1, 4)).transpose(1, 0, 2).reshape(128, NRB * 256)
    ssR = np.tile(fs[pos], (1, 1, 4)).transpose(1, 0, 2).reshape(128, NRB * 256)
    ccR = np.ascontiguousarray(ccR).astype(bf)
    ssR = np.ascontiguousarray(ssR).astype(bf)

    trim = np.where(
        np.arange(128)[:, None] > np.arange(128)[None, :], -1e30, 0.0
    ).astype(np.float32)
    iden = np.eye(128, dtype=bf)

    in_maps = []
    for c in range(NC):
        wqkv = np.concatenate(
            [
                wqTp[:, 4 * c: 4 * c + 4].reshape(D, 512),
                wkTp[:, c],
                wvT[:, c],
            ],
            axis=1,
        ).astype(bf)
        in_maps.append(
            {
                "xT": xT,
                "wqkvT": np.ascontiguousarray(wqkv),
                "woT": woT,
                "ccR": ccR,
                "ssR": ssR,
                "trim": trim,
                "iden": iden,
            }
        )

    res = bass_utils.run_bass_kernel_spmd(
        nc, in_maps, core_ids=list(range(NC)), trace=PROFILE, tmpdir=TMPDIR
    )
    if PROFILE:
        print(f"HW exec time: {res.exec_time_ns} ns")
        if res.instructions_and_trace is not None:
            print(f"trace: {res.instructions_and_trace[1]}")

    out_full = np.empty((BS, D), dtype=np.float32)
    for c in range(NC):
        out_full[R * c: R * (c + 1)] = res.results[c]["out"]
    return out_full.reshape(B, S, D)


# revision 13
# speedup vs baseline: 1.2205x; 1.0676x over previous
"""Distributed Trainium2 attention kernel (8 NeuronCores).

Strategy: tensor-parallel over heads for QKV projection + attention
(4 query heads + their 1 shared KV head per core), then an AllToAll
switches to row-sharding so each core computes the output projection for
its 512 rows with the full wo. Host reassembles rows. All matmuls run in
bf16 with fp32 PSUM accumulation.

The PE sustains ~0.5 ns/moving-column (2.0 GHz effective under the
chip-wide power state) with LDWEIGHTS and semaphore updates fully hidden,
so the design minimizes *streamed columns* and keeps every other engine
off the PE's critical path:

- Phase B (QKV+RoPE): x tiles stationary, wqkv moving; all of x-group
  0 and 1 plus the weights are DMA'd up front (the JIT issue scheme
  starved the PE for ~30us at startup via engine-FIFO head-of-line
  blocking).
- Phase C (attention, [keys, q] layout): score tiles for consecutive
  key-block PAIRS are packed contiguously into one 2-bank PSUM tile so
  ONE exp instruction covers both (halves ScalarE's ~300ns/op fixed
  cost; ScalarE is the phase C bottleneck). The softmax denominator is
  DVE/GpSimd-accumulated from the exp'd tiles (whole chains alternate
  engines; the adds of one chain are serial anyway). The chain tail is
  one ones-stationary rowsum matmul ([1,512] = denominators), a [1,512]
  DVE reciprocal, and the ones^T (x) recip broadcast matmul -- no PE
  transposes and no ScalarE involvement.
- Phase D (output projection): two 4-bank PSUM sets; each cg's last
  head-group (the heads whose AllToAll lands last) is deferred until
  after the NEXT cg's first 24 head-tiles are emitted, so the PE always
  has ~25us of runway while the final AllToAll is in flight.

RoPE is applied in row-major layout via a host-side even/odd column
permutation of wq/wk (rotation becomes contiguous half-block arithmetic),
then q/k are transposed to [head_dim, rows] on the TensorEngine.
"""

import numpy as np
import ml_dtypes

import concourse.bass as bass
import concourse.mybir as mybir
import concourse.tile as tile
from concourse import bacc
from concourse import bass_utils

B, S, D = 2, 2048, 4096
H, HKV, HD = 32, 8, 128
HD2 = HD // 2
NC = 8
HL = H // NC            # 4 local q heads per core
BS = B * S              # 4096 global rows
R = BS // NC            # 512 output rows per core
NRB = BS // 128         # 32 row blocks
NDT = D // 128          # 32 contraction tiles
SCALE = 1.0 / float(np.sqrt(HD))
BF = mybir.dt.bfloat16
F32 = mybir.dt.float32

PROFILE = False         # set by test.py for neuron-profile capture
TMPDIR = None           # set by test.py to keep the trace dir


def _emit(nc, tc, io):
    xT, wqkvT, woT, ccR, ssR, trim, iden, out = io

    engs3 = (nc.sync, nc.scalar, nc.gpsimd)

    with (
        tc.tile_pool(name="cbuf", bufs=1) as cbuf,
        tc.tile_pool(name="qbuf", bufs=1) as qbuf,
        tc.tile_pool(name="kvbuf", bufs=1) as kvbuf,
        tc.tile_pool(name="dram", bufs=1, space="DRAM") as dram,
        tc.tile_pool(name="ps", bufs=1, space="PSUM") as ps,
    ):
        # ---- long-lived SBUF state ----
        q_sb = qbuf.tile([128, HL * BS], BF, tag="q")     # col = h*4096 + row
        kT_sb = kvbuf.tile([128, BS], BF, tag="k")        # col = row
        v_sb = kvbuf.tile([128, BS], BF, tag="v")         # col = rb*128 + hd

        trim_sb = cbuf.tile([128, 128], F32, tag="tm")
        iden_sb = cbuf.tile([128, 128], BF, tag="idn")
        onec_sb = cbuf.tile([128, 1], BF, tag="onec")
        oner_sb = cbuf.tile([1, 128], BF, tag="oner")

        # one AllToAll per local head (fired as soon as that head's chains
        # drain) so phase D's inputs arrive progressively
        a2a_in = [dram.tile([BS // 4, R], BF, name=f"a2a_in{h}") for h in range(4)]
        a2a_out = [dram.tile([BS // 4, R], BF, name=f"a2a_out{h}") for h in range(4)]

        # ================= phase B: QKV projection + RoPE =================
        with (
            tc.tile_pool(name="wbuf", bufs=1) as wbuf,
            tc.tile_pool(name="xs", bufs=1) as xs,
            tc.tile_pool(name="cs", bufs=6) as cs,
            tc.tile_pool(name="ts", bufs=8) as ts,
        ):
            # resident QKV weights: col = dt*768 + [0:512 q | 512:640 k | 640:768 v]
            w_sb = wbuf.tile([128, NDT * 768], BF, tag="w")
            # x tiles: one [128, 4*512] quad covers 4 d-slices x 512 rows
            xg = [[None] * (NDT // 4) for _ in range(8)]

            def issue_xg(g, dq):
                t = xs.tile([128, 2048], BF, tag="x", bufs=16, name=f"x{g}_{dq}")
                src_ap = xT[dq * 512:(dq + 1) * 512, g * 512:(g + 1) * 512] \
                    .rearrange("(b p) c -> p b c", p=128)
                dst_ap = t[:].rearrange("p (b c) -> p b c", b=4)
                engs3[(g * 8 + dq + 1) % 3].dma_start(dst_ap, src_ap)
                xg[g][dq] = t

            def issue_w4(q):
                # one DMA covers 4 contraction tiles (1536B/partition/tile)
                src_ap = wqkvT[q * 512:(q + 1) * 512, :] \
                    .rearrange("(d p) c -> p d c", p=128)
                dst_ap = w_sb[:, q * 3072: (q + 1) * 3072] \
                    .rearrange("p (d c) -> p d c", d=4)
                engs3[q % 3].dma_start(dst_ap, src_ap)

            nc.sync.dma_start(trim_sb[:], trim[:])
            nc.scalar.dma_start(iden_sb[:], iden[:])
            nc.vector.memset(onec_sb[:], 1.0)
            nc.vector.memset(oner_sb[:], 1.0)
            # Everything the first two row-groups need is issued up front
            # with no buffer-recycle waits at the head of any engine FIFO:
            # weights (batched 4 tiles per DMA) interleaved with x-group 0,
            # then x-group 1 (xs has exactly 16 slots).
            issue_w4(0)
            issue_xg(0, 0)
            for q in range(1, 8):
                issue_w4(q)
                issue_xg(0, q)

            # rope tables: one [128, 1024] tile covers 4 row blocks
            csq = {}

            def issue_cs(q):
                cct = cs.tile([128, 1024], BF, tag="cc", bufs=3, name=f"cc{q}")
                engs3[q % 3].dma_start(cct[:], ccR[:, q * 1024: (q + 1) * 1024])
                sst = cs.tile([128, 1024], BF, tag="ss", bufs=3, name=f"ss{q}")
                engs3[(q + 1) % 3].dma_start(sst[:], ssR[:, q * 1024: (q + 1) * 1024])
                csq[q] = (cct, sst)

            issue_cs(0)
            for dq in range(8):
                issue_xg(1, dq)

            # rope tails are emitted one rb late, behind rb+1's matmuls
            def b_rope_tail_q(rb, ps_q):
                if rb % 4 == 0 and rb // 4 + 1 < 8:
                    issue_cs(rb // 4 + 1)
                cq, sq = csq[rb // 4]
                cct = cq[:, (rb % 4) * 256: (rb % 4 + 1) * 256]
                sst = sq[:, (rb % 4) * 256: (rb % 4 + 1) * 256]

                qe = ps_q[:].rearrange("p (h d) -> p h d", d=128)[:, :, 0:HD2]
                qo = ps_q[:].rearrange("p (h d) -> p h d", d=128)[:, :, HD2:HD]
                t1 = ts.tile([128, 256], BF, tag="t")
                t2 = ts.tile([128, 256], BF, tag="t")
                t3 = ts.tile([128, 256], BF, tag="t")
                t4 = ts.tile([128, 256], BF, tag="t")
                nc.vector.tensor_mul(t1[:], qe, cct)
                nc.vector.tensor_mul(t2[:], qo, sst)
                nc.vector.tensor_mul(t3[:], qe, sst)
                nc.vector.tensor_mul(t4[:], qo, cct)
                qrot = ts.tile([128, 512], BF, tag="qr")
                qre = qrot[:].rearrange("p (h d) -> p h d", d=128)[:, :, 0:HD2]
                qro = qrot[:].rearrange("p (h d) -> p h d", d=128)[:, :, HD2:HD]
                nc.vector.tensor_sub(qre, t1[:], t2[:])
                nc.vector.tensor_add(qro, t3[:], t4[:])
                return (qrot, cct, sst)

            def b_transpose_tail_q(rb, qrot):
                ps_tq = ps.tile([128, 512], BF, tag="aux", bufs=1, padded_shape=[128, 1024])
                for h in range(HL):
                    nc.tensor.transpose(
                        ps_tq[:, h * 128: (h + 1) * 128],
                        qrot[:, h * 128: (h + 1) * 128],
                        iden_sb[:],
                    )
                q_dst = (
                    q_sb[:]
                    .rearrange("p (h r) -> p h r", h=HL)
                    [:, :, rb * 128: (rb + 1) * 128]
                )
                nc.vector.tensor_copy(
                    q_dst, ps_tq[:].rearrange("p (h r) -> p h r", h=HL)
                )

            def b_rope_tail_kv(rb, ps_kv, cct, sst):
                ke = ps_kv[:, 0:HD2]
                ko = ps_kv[:, HD2:HD]
                u1 = ts.tile([128, 64], BF, tag="u")
                u2 = ts.tile([128, 64], BF, tag="u")
                u3 = ts.tile([128, 64], BF, tag="u")
                u4 = ts.tile([128, 64], BF, tag="u")
                nc.vector.tensor_mul(u1[:], ke, cct[:, 0:HD2])
                nc.vector.tensor_mul(u2[:], ko, sst[:, 0:HD2])
                nc.vector.tensor_mul(u3[:], ke, sst[:, 0:HD2])
                nc.vector.tensor_mul(u4[:], ko, cct[:, 0:HD2])

                krot = ts.tile([128, 128], BF, tag="kr")
                nc.vector.tensor_sub(krot[:, 0:HD2], u1[:], u2[:])
                nc.vector.tensor_add(krot[:, HD2:HD], u3[:], u4[:])

                # v: plain copy to row-major storage
                nc.scalar.activation(
                    v_sb[:, rb * 128: (rb + 1) * 128], ps_kv[:, 128:256],
                    mybir.ActivationFunctionType.Copy,
                )
                return (krot,)

            def b_transpose_tail_kv(rb, krot):
                ps_tk = ps.tile([128, 128], BF, tag="aux", bufs=1, padded_shape=[128, 1024])
                nc.tensor.transpose(ps_tk[:], krot[:], iden_sb[:])
                nc.vector.tensor_copy(kT_sb[:, rb * 128: (rb + 1) * 128], ps_tk[:])

            pending = None
            rot = None
            for rb in range(NRB):
                g, ri = rb // 4, rb % 4
                ps_q = ps.tile([128, 512], F32, tag="pa", bufs=3)
                ps_kv = ps.tile([128, 256], F32, tag="s2", bufs=2, padded_shape=[128, 1024])
                for dt in range(NDT):
                    xt = xg[g][dt // 4][:, (dt % 4) * 512 + ri * 128:
                                        (dt % 4) * 512 + (ri + 1) * 128]
                    st, sp = dt == 0, dt == NDT - 1
                    nc.tensor.matmul(
                        ps_q[:], xt, w_sb[:, dt * 768: dt * 768 + 512],
                        start=st, stop=sp,
                    )
                    nc.tensor.matmul(
                        ps_kv[:], xt, w_sb[:, dt * 768 + 512: dt * 768 + 768],
                        start=st, stop=sp,
                    )
                    # prefetch next row-group's x quads, spread over this group
                    # (groups 0 and 1 were fully issued up front)
                    if ri == 2 and g >= 1 and g + 1 < 8 and dt % 4 == 1:
                        issue_xg(g + 1, dt // 4)
                    if dt == 2 and pending is not None:
                        pq = b_rope_tail_q(pending[0], pending[1])
                        pkv = b_rope_tail_kv(pending[0], pending[2], pq[1], pq[2])
                        rot = (pending[0], pq[0]) + pkv
                        pending = None
                    if dt == 12 and rot is not None:
                        b_transpose_tail_q(rot[0], rot[1])
                        b_transpose_tail_kv(rot[0], rot[2])
                        rot = None
                pending = (rb, ps_q, ps_kv)
            pq = b_rope_tail_q(pending[0], pending[1])
            pkv = b_rope_tail_kv(pending[0], pending[2], pq[1], pq[2])
            b_transpose_tail_q(pending[0], pq[0])
            b_transpose_tail_kv(pending[0], pkv[0])

        # ============ phase C: causal attention (flipped PV) ============
        with (
            tc.tile_pool(name="es", bufs=6) as es,
            tc.tile_pool(name="rns", bufs=4) as rns,
            tc.tile_pool(name="abuf", bufs=1) as abuf,
            tc.tile_pool(name="ws", bufs=1) as ws,
            tc.tile_pool(name="osp", bufs=4) as osp,
        ):
            at_sb = abuf.tile([128, 32 * 512], BF, tag="at")  # col = ht*512+row
            # head-major so the earliest AllToAlls feed phase D's first
            # accumulation steps
            ht_order = [4 * i + l for l in range(4) for i in range(8)]
            wt0 = {}  # prefetched wo tiles for cg 0

            if True:

                def head_done(h):
                    """Fire head h's AllToAll + phase-D prefetches."""
                    nc.gpsimd.collective_compute(
                        "AllToAll",
                        mybir.AluOpType.bypass,
                        replica_groups=[list(range(NC))],
                        ins=[a2a_in[h].opt()],
                        outs=[a2a_out[h].opt()],
                    )
                    dst_ap = at_sb[:].rearrange(
                        "p (i c) -> p i c", c=512
                    )[:, h::4, :]
                    src_ap = a2a_out[h][:].rearrange("(i p) c -> p i c", p=128)
                    nc.gpsimd.dma_start(dst_ap, src_ap)
                    if h == 0:
                        for k in range(0, 32, 4):
                            wt = ws.tile([128, 2048], BF, tag="wo", bufs=16,
                                         name=f"wt0_{k}")
                            i0, lv = k % 8, k // 8
                            src_ap = woT[:].rearrange(
                                "(a l p) c -> p a l c", p=128, l=4
                            )[:, i0: i0 + 4, lv, 0:512]
                            nc.gpsimd.dma_start(
                                wt[:].rearrange("p (b c) -> p b c", b=4), src_ap
                            )
                            for n, ht in enumerate(ht_order[k: k + 4]):
                                wt0[ht] = wt[:, n * 512: (n + 1) * 512]

                def attn_chain(b, h, ci):
                    # pair-packed j pipeline: two key blocks' score tiles land
                    # contiguously in one 2-bank PSUM tile -> one exp, then PV
                    # (v-stationary, [hd, q] PSUM accumulation) with the
                    # softmax denominator accumulated from the exp'd tiles
                    # (vector/gpsimd alternating by chain).
                    qbase = h * BS + b * S
                    jmax = 4 * ci + 3
                    ps_attn = ps.tile([128, 512], F32, tag="pa", bufs=3,
                                       name=f"pa{b}{h}{ci}")
                    # Denominator strategy: each pair's exp'd tiles are merged
                    # into a PARTIAL tile with a single DVE op (off-diagonal:
                    # one scalar_tensor_tensor et_a+et_b; diagonal: copy+add at
                    # their distinct q offsets); the tail's PE rowsum matmuls
                    # then accumulate Sum_keys over all partials directly in a
                    # [1,512] PSUM bank. Trades idle PE columns for scarce DVE
                    # time.
                    partials = []  # (tile, col_lo, col_hi)

                    def pv_and_acc(p, et2, specs):
                        for (j, off, w, qo) in specs:
                            kcol = (b * 16 + j) * 128
                            nc.tensor.matmul(
                                ps_attn[:, qo: qo + w],
                                v_sb[:, kcol: kcol + 128],
                                et2[:, off: off + w],
                                start=(j == 0), stop=(j == jmax),
                                skip_group_check=True,
                            )
                        (ja, offa, wa, qoa), (jb, offb, wb, qob) = specs
                        part = rns.tile([128, 512], BF, tag="part", bufs=18,
                                        name=f"pt{b}{h}{ci}_{p}")
                        if wa == 512 and wb == 512:
                            nc.vector.scalar_tensor_tensor(
                                part[:], et2[:, 0:512], 1.0,
                                et2[:, 512:1024],
                                op0=mybir.AluOpType.mult,
                                op1=mybir.AluOpType.add,
                            )
                            partials.append((part, 0, 512))
                        else:
                            nc.vector.tensor_copy(
                                part[:, qoa: qoa + wa], et2[:, offa: offa + wa]
                            )
                            nc.vector.tensor_add(
                                part[:, qob: qob + wb], part[:, qob: qob + wb],
                                et2[:, offb: offb + wb],
                            )
                            partials.append((part, qoa, 512))

                    prev = None
                    for p in range(2 * ci + 2):
                        ja, jb = 2 * p, 2 * p + 1
                        q0a = max(ja * 128, 512 * ci)
                        wa = 512 * ci + 512 - q0a
                        q0b = max(jb * 128, 512 * ci)
                        wb = 512 * ci + 512 - q0b
                        qoa, qob = q0a - 512 * ci, q0b - 512 * ci
                        pp = ps.tile([128, 1024], F32, tag="s2", bufs=2,
                                      name=f"s{b}{h}{ci}_{p}")
                        nc.tensor.matmul(
                            pp[:, 0:wa],
                            kT_sb[:, (b * 16 + ja) * 128: (b * 16 + ja) * 128 + 128],
                            q_sb[:, qbase + q0a: qbase + q0a + wa],
                            start=True, stop=True, skip_group_check=True,
                        )
                        # region b needs its own has_written clear only when
                        # it lands in bank 1; when it shares bank 0 with
                        # region a (wa < 512), region a's start already
                        # cleared its bits and a second clear is not needed.
                        nc.tensor.matmul(
                            pp[:, wa: wa + wb],
                            kT_sb[:, (b * 16 + jb) * 128: (b * 16 + jb) * 128 + 128],
                            q_sb[:, qbase + q0b: qbase + q0b + wb],
                            start=(wa == 512), stop=True, skip_group_check=True,
                        )
                        if ja >= 4 * ci:
                            nc.vector.tensor_add(
                                pp[:, 0:128], pp[:, 0:128], trim_sb[:]
                            )
                        if jb >= 4 * ci:
                            nc.vector.tensor_add(
                                pp[:, wa: wa + 128], pp[:, wa: wa + 128],
                                trim_sb[:],
                            )
                        et2 = es.tile([128, 1024], BF, tag="e", bufs=6,
                                      name=f"e{b}{h}{ci}_{p}")
                        nc.scalar.activation(
                            et2[:, 0: wa + wb], pp[:, 0: wa + wb],
                            mybir.ActivationFunctionType.Exp, scale=SCALE,
                        )
                        if prev is not None:
                            pv_and_acc(*prev)
                        prev = (p, et2,
                                ((ja, 0, wa, qoa), (jb, wa, wb, qob)))
                        yield
                    pv_and_acc(*prev)

                    # tail: ones-stationary rowsum matmuls accumulate the
                    # denominators over all partials into a [1,512] PSUM bank
                    # (the first partial always spans the full 512 q), then a
                    # fast [1,512] DVE reciprocal, a scalar f32->bf16 copy,
                    # the ones^T (x) recip broadcast matmul and one
                    # normalizing multiply.
                    psr = ps.tile([1, 512], F32, tag="aux", bufs=1,
                                   padded_shape=[128, 512],
                                   name=f"rs{b}{h}{ci}")
                    for np_, (part, lo, hi) in enumerate(partials):
                        nc.tensor.matmul(
                            psr[0:1, lo:hi], onec_sb[:], part[:, lo:hi],
                            start=(np_ == 0), stop=(np_ == len(partials) - 1),
                            skip_group_check=True,
                        )
                    rcf = rns.tile([1, 512], F32, tag="rcf", bufs=2)
                    nc.vector.reciprocal_approx_fast(rcf[:], psr[:])
                    rc_row = rns.tile([1, 512], BF, tag="rcrow", bufs=2)
                    nc.scalar.activation(
                        rc_row[:], rcf[:], mybir.ActivationFunctionType.Copy
                    )
                    bc_ps = ps.tile([128, 512], F32, tag="aux", bufs=1,
                                     name=f"bc{b}{h}{ci}")
                    nc.tensor.matmul(
                        bc_ps[:], oner_sb[:], rc_row[:],
                        start=True, stop=True, skip_group_check=True,
                    )
                    bc = rns.tile([128, 512], F32, tag="bc", bufs=2)
                    nc.vector.tensor_copy(bc[:], bc_ps[:])
                    an = rns.tile([128, 512], BF, tag="an", bufs=6)
                    nc.vector.tensor_mul(an[:], ps_attn[:], bc[:])
                    nc.sync.dma_start(
                        a2a_in[h][128 * (b * 4 + ci): 128 * (b * 4 + ci) + 128, :],
                        an[:],
                    )
                    yield

                # continuous 2-in-flight worklist; fire each head's AllToAll
                # the moment its last chain drains
                todo = [(b, h, ci)
                        for h in range(4) for b in range(B)
                        for ci in (0, 3, 1, 2)]
                todo.reverse()
                left = {h: 2 * 4 for h in range(4)}
                active = [[todo[-1][1], attn_chain(*todo.pop())],
                          [todo[-1][1], attn_chain(*todo.pop())],
                          [todo[-1][1], attn_chain(*todo.pop())]]
                while active:
                    for ent in list(active):
                        if next(ent[1], StopIteration) is StopIteration:
                            active.remove(ent)
                            left[ent[0]] -= 1
                            if left[ent[0]] == 0:
                                head_done(ent[0])
                            if todo:
                                active.append(
                                    [todo[-1][1], attn_chain(*todo.pop())]
                                )

            # ======== phase D: output projection for this core's rows ========
            # Two 4-bank PSUM sets; each cg's k=6,7 head-groups (the heads
            # whose AllToAll lands last) are deferred until after the next
            # cg's k=0..5, so the final AllToAll is fully hidden.
            if True:
                wtq = [None] * 8  # per-cg {ht: wt slice}

                def d_load_wt(cg, k):
                    wq4 = ws.tile([128, 2048], BF, tag="wo",
                                  bufs=16, name=f"wt{cg}_{k}")
                    i0, lv = 4 * (k % 2), k // 2
                    src_ap = woT[:].rearrange(
                        "(a l p) c -> p a l c", p=128, l=4
                    )[:, i0: i0 + 4, lv,
                      cg * 512: (cg + 1) * 512]
                    engs3[k % 3].dma_start(
                        wq4[:].rearrange("p (b c) -> p b c", b=4),
                        src_ap,
                    )
                    for n, ht2 in enumerate(ht_order[4 * k: 4 * k + 4]):
                        wtq[cg][ht2] = wq4[:, n * 512: (n + 1) * 512]

                po_sets = [None] * 8

                def d_alloc(cg):
                    if cg % 2 == 0:
                        t = [ps.tile([128, 512], F32, tag="pa", bufs=3,
                                     name=f"po{cg}_{i}")[:] for i in range(3)]
                        t.append(ps.tile([128, 512], F32, tag="aux", bufs=1,
                                         name=f"po{cg}_3")[:])
                        return t
                    s0 = ps.tile([128, 1024], F32, tag="s2", bufs=2,
                                 name=f"po{cg}_01")
                    s1 = ps.tile([128, 1024], F32, tag="s2", bufs=2,
                                 name=f"po{cg}_23")
                    return [s0[:, 0:512], s0[:, 512:1024],
                            s1[:, 0:512], s1[:, 512:1024]]

                def d_emit(cg, ks):
                    po = po_sets[cg]
                    for k in ks:
                        if cg == 0:
                            wt4 = [wt0[ht] for ht in ht_order[4 * k: 4 * k + 4]]
                        else:
                            d_load_wt(cg, k)
                            wt4 = [wtq[cg][ht] for ht in ht_order[4 * k: 4 * k + 4]]
                        for n, ht in enumerate(ht_order[4 * k: 4 * k + 4]):
                            n_ht = 4 * k + n
                            for rt in range(4):
                                nc.tensor.matmul(
                                    po[rt],
                                    at_sb[:, ht * 512 + rt * 128:
                                          ht * 512 + (rt + 1) * 128],
                                    wt4[n],
                                    start=(n_ht == 0), stop=(n_ht == 31),
                                    skip_group_check=True,
                                )

                def d_copyout(cg):
                    for rt in range(4):
                        ot = osp.tile([128, 512], F32, tag="o")
                        nc.vector.tensor_copy(ot[:], po_sets[cg][rt])
                        engs3[rt % 2].dma_start(
                            out[rt * 128: (rt + 1) * 128,
                                cg * 512: (cg + 1) * 512],
                            ot[:],
                        )
                    po_sets[cg] = None

                for cg in range(9):
                    if cg < 8:
                        wtq[cg] = {}
                        po_sets[cg] = d_alloc(cg)
                        d_emit(cg, range(6))
                    if cg >= 1:
                        d_emit(cg - 1, range(6, 8))
                        d_copyout(cg - 1)


def _build():
    nc = bacc.Bacc("TRN2", target_bir_lowering=False, debug=False, num_devices=NC)
    xT = nc.dram_tensor("xT", [D, BS], BF, kind="ExternalInput")
    wqkvT = nc.dram_tensor("wqkvT", [D, 768], BF, kind="ExternalInput")
    woT = nc.dram_tensor("woT", [D, D], BF, kind="ExternalInput")
    ccR = nc.dram_tensor("ccR", [128, NRB * 256], BF, kind="ExternalInput")
    ssR = nc.dram_tensor("ssR", [128, NRB * 256], BF, kind="ExternalInput")
    trim = nc.dram_tensor("trim", [128, 128], F32, kind="ExternalInput")
    iden = nc.dram_tensor("iden", [128, 128], BF, kind="ExternalInput")
    out = nc.dram_tensor("out", [R, D], F32, kind="ExternalOutput")
    with tile.TileContext(nc) as tc:
        _emit(nc, tc, (xT, wqkvT, woT, ccR, ssR, trim, iden, out))
    nc.compile()
    return nc


_NC = None


def kernel(x, wq, wk, wv, wo, freqs_cos, freqs_sin, mask, start_pos):
    global _NC
    if _NC is None:
        _NC = _build()
    nc = _NC
    bf = ml_dtypes.bfloat16

    x = np.asarray(x, dtype=np.float32)
    xT = np.ascontiguousarray(x.reshape(BS, D).T).astype(bf)

    perm = np.concatenate([np.arange(0, HD, 2), np.arange(1, HD, 2)])
    wqTp = np.asarray(wq, np.float32).T.reshape(D, H, HD)[:, :, perm]
    wkTp = np.asarray(wk, np.float32).T.reshape(D, HKV, HD)[:, :, perm]
    wvT = np.asarray(wv, np.float32).T.reshape(D, HKV, HD)
    woT = np.ascontiguousarray(np.asarray(wo, np.float32).T).astype(bf)

    fc = np.asarray(freqs_cos, np.float32)
    fs = np.asarray(freqs_sin, np.float32)
    # row-major RoPE tables per row block, replicated x4 along free axis
    pos = (np.arange(BS) % S).reshape(NRB, 128)
    ccR = np.tile(fc[pos], (1, 1, 4)).transpose(1, 0, 2).reshape(128, NRB * 256)
    ssR = np.tile(fs[pos], (1, 1, 4)).transpose(1, 0, 2).reshape(128, NRB * 256)
    ccR = np.ascontiguousarray(ccR).astype(bf)
    ssR = np.ascontiguousarray(ssR).astype(bf)

    trim = np.where(
        np.arange(128)[:, None] > np.arange(128)[None, :], -1e30, 0.0
    ).astype(np.float32)
    iden = np.eye(128, dtype=bf)

    in_maps = []
    for c in range(NC):
        wqkv = np.concatenate(
            [
                wqTp[:, 4 * c: 4 * c + 4].reshape(D, 512),
                wkTp[:, c],
                wvT[:, c],
            ],
            axis=1,
        ).astype(bf)
        in_maps.append(
            {
                "xT": xT,
                "wqkvT": np.ascontiguousarray(wqkv),
                "woT": woT,
                "ccR": ccR,
                "ssR": ssR,
                "trim": trim,
                "iden": iden,
            }
        )

    res = bass_utils.run_bass_kernel_spmd(
        nc, in_maps, core_ids=list(range(NC)), trace=PROFILE, tmpdir=TMPDIR
    )
    if PROFILE:
        print(f"HW exec time: {res.exec_time_ns} ns")
        if res.instructions_and_trace is not None:
            print(f"trace: {res.instructions_and_trace[1]}")

    out_full = np.empty((BS, D), dtype=np.float32)
    for c in range(NC):
        out_full[R * c: R * (c + 1)] = res.results[c]["out"]
    return out_full.reshape(B, S, D)
